# revision 27
# baseline (speedup 1.0000x reference)
"""Multi-head self-attention (RoPE, causal) Trainium2 Bass kernel.

Problem: B=4, S=2048, D=1024, H=16 heads, hd=64, fused QKV + RoPE +
causal softmax attention + output projection (torch-Linear convention).

Sharding: Megatron-style tensor parallel over heads. Each of the 8
NeuronCores owns 2 heads: it projects the full token stream through its
128-row slices of Wq/Wk/Wv, applies RoPE, runs causal attention for its
2 heads x 4 batches, and computes a partial output projection
h_core @ Wo[:, core_slice].T  (transposed layout). The host sums the 8
partial outputs and adds the output bias.

v4 design notes (vs v2 baseline at ~601us; this version ~490us):
  - RoPE cos/sin tables are computed host-side in build_in_maps from the
    positions input (fp64 numpy -> bf16) and DMA'd in. This removed the
    on-device range-reduced Sin pipeline (22us of DVE at startup + 16
    activations) and cut the end-to-end relative error 1.8% -> 0.76%
    (the bf16 Sin spline was the dominant error source).
  - Score psum groups are 1024 wide (2 banks, double buffered = 4
    banks); each exp ACTIVATE covers 1024 columns, halving scalar-engine
    per-instruction overhead (136 calls).
  - Softmax denominator path: den rows gather to a q-ordered [16,128]
    fp16 tile, fp32 reciprocal_approx_fast in that layout, one
    partition-major DMA linearizes to rec_row[1,S] fp16, and a K=1
    ones-matmul broadcasts 1/den to [64,512] psum (rides the pv psum
    pool). This replaced the ejs/idf selector matmuls (128 MMs), a PE
    transpose, and two dedicated psum banks.
  - P@V staging: one [128,512] fp16 CAST per chunk captures h values
    AND den rows; hu/den are views into it.
  - Emission scheduling: proj chunk pieces are decoupled
    ([mm q][mm k][rope q][rope k][v]) so each psum group's DVE cast has
    ~2 pieces of slack before its consumer matmul; fillers (proj of the
    next batch, P@V of the previous problem, o-proj) live in a queue
    that drains proportionally between score windows and carries across
    iterations, keeping the PE fed so the HAM clock gate stays warm.
  - RoPE final add and the causal masks run on gpsimd; o-proj and P@V
    psum->sbuf casts run on DVE; the scalar engine runs only Exp.
  - qT/kT residents rotate through 2 buffers; x chunk 0 is DMA'd in 8
    per-ktile slices so the first projection matmul starts ~5us earlier.
"""

import os
import sys

for _p in ("/opt/trn_rl_repo",):
    if os.path.isdir(_p) and _p not in sys.path:
        sys.path.append(_p)

import math

import ml_dtypes
import numpy as np

import concourse.bass as bass
import concourse.mybir as mybir
import concourse.tile as tile
from concourse import bacc
from concourse.bass import ts, ds
from concourse.bass_utils import run_bass_kernel_spmd

BF16 = ml_dtypes.bfloat16

B = 4
S = 2048
D = 1024
H = 16
HD = 64
NCORES = 8
HPC = H // NCORES          # heads per core = 2
PC = HPC * HD              # partition rows per core's heads = 128
T = B * S                  # 8192 tokens
KT = D // 128              # f_in k-tiles = 8
NTOK = T // 128            # 64 token tiles of 128
SCALE = 1.0 / math.sqrt(HD)
ROPE_THETA = 10000.0

TWO_PI = 2.0 * math.pi
INV_2PI = 1.0 / TWO_PI
MAGIC = 12582912.0         # 1.5 * 2**23, float32 round-to-nearest trick
HALF_PI = math.pi / 2.0

NQT = S // 128             # 16 q/k tiles per sequence
# triangular packing offsets for P_T: row kt covers q in [kt*128, S)
OFFS = [0] * NQT
for _kt in range(1, NQT):
    OFFS[_kt] = OFFS[_kt - 1] + (S - (_kt - 1) * 128)
PTRI_W = OFFS[-1] + (S - (NQT - 1) * 128)   # 17408

TC = 512                   # token chunk for projections
NTC = T // TC              # 16
CPB = S // TC              # proj chunks per batch = 4
QC = 512                   # P@V q-chunk width
NPV = S // QC              # P@V chunks per problem = 4
SGW = 512                  # scores psum group width (1 bank)


def _build_nc():
    nc = bacc.Bacc("TRN2", target_bir_lowering=False, debug=False,
                   num_devices=NCORES)
    dt = mybir.dt

    # ---- I/O ----
    x_in = nc.dram_tensor("x", [NTC, 128, KT * TC], dt.bfloat16,
                          kind="ExternalInput")
    cos_in = nc.dram_tensor("cost", [128, S], dt.bfloat16, kind="ExternalInput")
    sin_in = nc.dram_tensor("sint", [128, S], dt.bfloat16, kind="ExternalInput")
    wq_in = nc.dram_tensor("wq", [D, PC], dt.bfloat16, kind="ExternalInput")
    wk_in = nc.dram_tensor("wk", [D, PC], dt.bfloat16, kind="ExternalInput")
    wv_in = nc.dram_tensor("wv", [D, PC], dt.bfloat16, kind="ExternalInput")
    wo_in = nc.dram_tensor("wo", [PC, D], dt.bfloat16, kind="ExternalInput")
    bq_in = nc.dram_tensor("bq", [PC], dt.float32, kind="ExternalInput")
    bk_in = nc.dram_tensor("bk", [PC], dt.float32, kind="ExternalInput")
    bv_in = nc.dram_tensor("bv", [PC], dt.float32, kind="ExternalInput")
    out_d = nc.dram_tensor("out", [KT, T // 512, 128, 512], dt.bfloat16,
                           kind="ExternalOutput")
    KDEBUG = os.environ.get("KDEBUG") == "1"
    if KDEBUG:
        dbg_q = nc.dram_tensor("dbg_q", [128, T], dt.bfloat16, kind="ExternalOutput")
        dbg_k = nc.dram_tensor("dbg_k", [128, T], dt.bfloat16, kind="ExternalOutput")
        dbg_h = nc.dram_tensor("dbg_h", [128, T], dt.bfloat16, kind="ExternalOutput")
        dbg_pt = nc.dram_tensor("dbg_pt", [128, PTRI_W], dt.bfloat16,
                                kind="ExternalOutput")
        dbg_rr = nc.dram_tensor("dbg_rr", [1, S], dt.float16,
                                kind="ExternalOutput")

    # ---- inline constants ----
    # RT = R.T where (R @ q)[2i] = -q[2i+1], (R @ q)[2i+1] = q[2i],
    # block-diagonal over the 2 stacked heads (structure identical).
    r = np.zeros((PC, PC), dtype=np.float32)
    for h in range(HPC):
        for i in range(HD // 2):
            r[h * HD + 2 * i, h * HD + 2 * i + 1] = -1.0
            r[h * HD + 2 * i + 1, h * HD + 2 * i] = 1.0
    rt_np = np.ascontiguousarray(r.T).astype(BF16)
    # causal mask for diagonal scoresT blocks: keep k_local <= q_local
    mask_np = np.tril(np.ones((128, 128), dtype=np.float32)).T.astype(BF16)
    rt_d = nc.inline_tensor(rt_np, "rt_c")
    ones64_np = np.ones((1, HD), dtype=np.float16)
    ones64_d = nc.inline_tensor(ones64_np, "ones64_c")
    mask_d = nc.inline_tensor(mask_np, "mask_c")

    fp32 = dt.float32
    bf16 = dt.bfloat16
    fp16 = dt.float16

    with tile.TileContext(nc) as tc:
        with (
            tc.tile_pool(name="consts", bufs=1) as consts,
            tc.tile_pool(name="resid", bufs=1) as resid,
            tc.tile_pool(name="xp", bufs=2) as xp,
            tc.tile_pool(name="work", bufs=2) as work,
            tc.tile_pool(name="vst", bufs=2) as vst,
            tc.tile_pool(name="stg", bufs=2) as stg,
            tc.tile_pool(name="ptri", bufs=2) as ptri_pool,
            tc.tile_pool(name="bigps", bufs=2, space="PSUM") as bigps,
            tc.tile_pool(name="pvps", bufs=2, space="PSUM") as pvps,
            tc.tile_pool(name="accps", bufs=4, space="PSUM") as accps,
        ):
            # ---- load constants / weights to SBUF ----
            # x chunk 0 DMA first so the PE can start ASAP; weights on
            # separate queues so wq doesn't queue behind everything.
            xt0 = xp.tile([128, KT, TC], bf16, tag="xt")
            for _kt in range(KT):
                nc.scalar.dma_start(out=xt0[:, _kt, :],
                                    in_=x_in[0, :, ds(_kt * TC, TC)])

            wq_sb = consts.tile([128, KT, PC], bf16, tag="wq")
            wk_sb = consts.tile([128, KT, PC], bf16, tag="wk")
            wv_sb = consts.tile([128, KT, PC], bf16, tag="wv")
            nc.sync.dma_start(
                out=wq_sb, in_=wq_in.ap().rearrange("(kt p) m -> p kt m", p=128))
            nc.gpsimd.dma_start(
                out=wk_sb, in_=wk_in.ap().rearrange("(kt p) m -> p kt m", p=128))
            nc.scalar.dma_start(
                out=wv_sb, in_=wv_in.ap().rearrange("(kt p) m -> p kt m", p=128))
            wo_sb = consts.tile([128, D], bf16, tag="wo")
            nc.gpsimd.dma_start(out=wo_sb, in_=wo_in[:, :])
            rt_sb = consts.tile([128, 128], bf16, tag="rt")
            nc.sync.dma_start(out=rt_sb, in_=rt_d[:, :])
            mask_sb = consts.tile([128, 128], bf16, tag="mask")
            nc.sync.dma_start(out=mask_sb, in_=mask_d[:, :])
            ones64f_sb = consts.tile([1, HD], fp16, tag="ones64")
            nc.sync.dma_start(out=ones64f_sb, in_=ones64_d[:, :])
            bq_sb = consts.tile([128, 1], fp32, tag="bq")
            nc.sync.dma_start(out=bq_sb, in_=bq_in.ap().rearrange("(p o) -> p o", o=1))
            bk_sb = consts.tile([128, 1], fp32, tag="bk")
            nc.sync.dma_start(out=bk_sb, in_=bk_in.ap().rearrange("(p o) -> p o", o=1))
            bv_sb = consts.tile([128, 1], fp32, tag="bv")
            nc.sync.dma_start(out=bv_sb, in_=bv_in.ap().rearrange("(p o) -> p o", o=1))

            # ---- residents ----
            # qT/kT rotate through 2 buffers (proj of batch b+1 overlaps
            # scores of batch b); hT stays per-batch (read by o-proj two
            # problems later).
            qkh = {}

            def get_qk(bb, which):
                if (bb, which) not in qkh:
                    t = resid.tile([128, S], bf16, tag=which, bufs=2,
                                   name=f"{which}{bb}")
                    qkh[(bb, which)] = t
                return qkh[(bb, which)]

            hT = []
            for bb in range(B):
                th = resid.tile([128, S], bf16, tag=f"hT{bb}")
                hT.append(th)
            # v natural as repeating [ones(64) | d_h0(64) | d_h1(64)]
            # 192-col blocks (plus one trailing ones block): head0's P@V
            # lhsT is [ones|d0] (den in psum rows 0-63, h in 64-127) and
            # head1's is [d1|ones-of-next-block] (h in 0-63, den 64-127) --
            # both plain contiguous 128-col slices.
            NTB = NTOK // B            # 16 tok tiles per batch
            VW = NTB * 192 + 64
            vN = []
            for bb in range(B):
                tv = resid.tile([128, VW], bf16, tag=f"vN{bb}")
                vN.append(tv)
                nc.vector.memset(
                    bass.AP(tensor=tv.tensor, offset=tv.offset,
                            ap=[tv.ap[0], [192, NTB + 1], [1, 64]]), 1.0)
            # RoPE cos/sin tables [128, S] bf16, computed host-side
            cos_sb = consts.tile([128, S], bf16, tag="cosT")
            nc.gpsimd.dma_start(out=cos_sb, in_=cos_in[:, :])
            sin_sb = consts.tile([128, S], bf16, tag="sinT")
            nc.gpsimd.dma_start(out=sin_sb, in_=sin_in[:, :])

            # ---- phase 1: QKV projection + RoPE, per token chunk ----
            xt_cache = {0: xt0}

            def get_xt(tci):
                if tci not in xt_cache:
                    xt = xp.tile([128, KT, TC], bf16, tag="xt")
                    nc.scalar.dma_start(out=xt.rearrange("p a b -> p (a b)"),
                                        in_=x_in[tci, :, :])
                    xt_cache[tci] = xt
                return xt_cache[tci]

            def proj_qk_mm(tci, which, st):
                """q/k projection matmuls for one 512-token chunk."""
                xt = get_xt(tci)
                if tci + 1 < NTC:
                    get_xt(tci + 1)
                w_sb, b_sb = ((wq_sb, bq_sb) if which == "q"
                              else (wk_sb, bk_sb))
                pa = accps.tile([128, TC], fp32, tag="acc")
                for kt in range(KT):
                    nc.tensor.matmul(pa, lhsT=w_sb[:, kt, :],
                                     rhs=xt[:, kt, :],
                                     start=(kt == 0), stop=(kt == KT - 1))
                a_sb = work.tile([128, TC], bf16, tag="a_sb")
                nc.vector.tensor_scalar_add(a_sb, pa, b_sb)
                st[which] = a_sb

            def proj_qk_rope(tci, which, st):
                """RoPE for the chunk: rotation matmul + combine."""
                dest = get_qk(tci // CPB, which)
                a_sb = st.pop(which)
                pb = accps.tile([128, TC], fp32, tag="acc")
                nc.tensor.matmul(pb, lhsT=rt_sb, rhs=a_sb,
                                 start=True, stop=True)
                ssl = ds((tci * TC) % S, TC)
                t1 = work.tile([128, TC], bf16, tag="t1")
                nc.vector.tensor_mul(t1, a_sb, cos_sb[:, ssl])
                t2 = work.tile([128, TC], bf16, tag="t2")
                nc.vector.tensor_mul(t2, pb, sin_sb[:, ssl])
                nc.gpsimd.tensor_add(dest[:, ts(tci % CPB, TC)], t1, t2)

            def proj_v(tci):
                """v projection, transposed production (wv stationary),
                then DMA-transposed into natural layout vN on sync."""
                xt = get_xt(tci)
                pv = accps.tile([128, TC], fp32, tag="acc")
                for kt in range(KT):
                    nc.tensor.matmul(pv, lhsT=wv_sb[:, kt, :],
                                     rhs=xt[:, kt, :],
                                     start=(kt == 0), stop=(kt == KT - 1))
                vTst = vst.tile([128, TC], bf16, tag="vTst")
                nc.vector.tensor_scalar_add(vTst, pv, bv_sb)
                for sub in range(TC // 128):
                    tl = (tci % CPB) * (TC // 128) + sub
                    nc.sync.dma_start_transpose(
                        vN[tci // CPB][:, ds(192 * tl + 64, 128)],
                        vTst[:, ts(sub, 128)])

            def proj_pieces(b):
                out = []
                for cc in range(CPB):
                    tci = b * CPB + cc
                    st = {}
                    out.append(lambda t=tci, s=st: proj_qk_mm(t, "q", s))
                    out.append(lambda t=tci, s=st: proj_qk_mm(t, "k", s))
                    out.append(lambda t=tci, s=st: proj_qk_rope(t, "q", s))
                    out.append(lambda t=tci, s=st: proj_qk_rope(t, "k", s))
                    out.append(lambda t=tci: proj_v(t))
                return out

            # ---- phase 2a: scores + exp + mask for one (batch, head) ----
            def score_pieces(i, pt):
                """Closures, each = one <=1024-col psum group: matmul the
                row pieces intersecting the window, exp into pt, mask
                any diagonal blocks fully inside the window."""
                b, h = i // HPC, i % HPC
                hsl = ds(h * HD, HD)
                kTb = get_qk(b, "k")
                qTb = get_qk(b, "q")
                pieces = []
                x0 = 0
                while x0 < PTRI_W:
                    w = min(SGW, PTRI_W - x0)

                    def piece(x0=x0, w=w):
                        sc = bigps.tile([128, SGW], fp32, tag="big")
                        # rows intersecting flat-span window [x0, x0+w)
                        for kt in range(NQT):
                            r0, r1 = OFFS[kt], OFFS[kt] + (S - kt * 128)
                            lo, hi = max(r0, x0), min(r1, x0 + w)
                            if lo >= hi:
                                continue
                            q0 = kt * 128 + (lo - r0)
                            # split at 512-aligned psum columns: a matmul
                            # output must not cross a PSUM bank boundary
                            cuts = [lo]
                            nb = (lo - x0) // 512 * 512 + 512 + x0
                            while nb < hi:
                                cuts.append(nb)
                                nb += 512
                            cuts.append(hi)
                            for aa, bb in zip(cuts, cuts[1:]):
                                nc.tensor.matmul(
                                    sc[:, ds(aa - x0, bb - aa)],
                                    lhsT=kTb[hsl, ds(kt * 128, 128)],
                                    rhs=qTb[hsl, ds(q0 + (aa - lo), bb - aa)],
                                    start=True, stop=True)
                        nc.scalar.activation(
                            pt[:, ds(x0, w)], sc[:, 0:w],
                            mybir.ActivationFunctionType.Exp, scale=SCALE)
                        for kt in range(NQT):
                            if x0 <= OFFS[kt] and OFFS[kt] + 128 <= x0 + w:
                                dsl = ds(OFFS[kt], 128)
                                nc.gpsimd.tensor_mul(
                                    pt[:, dsl], pt[:, dsl], mask_sb)

                    pieces.append(piece)
                    x0 += w
                return pieces

            # ---- phase 2b: P@V (V stationary) + normalize into hT ----
            # Each P@V chunk's psum acc [128,512] (den rows + h rows) is
            # cast once to fp16 staging; den rows gather into [16,128],
            # DMA-transpose to [128,16], DVE reciprocal -> fp16, one DMA
            # scatters to rec_row [1,S] (q-order), and a K=1 ones-matmul
            # broadcasts 1/den to [64,512] psum for the normalize mult.
            def pv_pieces(i, pt):
                b, h = i // HPC, i % HPC
                st = {}
                pieces = []

                def mm_piece(c):
                    q0 = c * QC
                    acc = pvps.tile([128, QC], fp32, tag="pv")
                    kts = [kt for kt in range(NQT) if kt * 128 < q0 + QC]
                    for j, kt in enumerate(kts):
                        lo = max(kt * 128, q0)
                        w = q0 + QC - lo
                        lt = vN[b][:, ds(192 * kt + 128 * h, 128)]
                        nc.tensor.matmul(
                            acc[:, ds(lo - q0, w)], lhsT=lt,
                            rhs=pt[:, ds(OFFS[kt] + lo - kt * 128, w)],
                            start=(j == 0), stop=(j == len(kts) - 1))
                    if c == 0:
                        d16 = stg.tile([16, 128], fp16, tag="den16")
                        st["den16"] = d16
                    sacc = stg.tile([128, QC], fp16, tag="sacc", bufs=5)
                    nc.vector.tensor_copy(sacc, acc)
                    st[("hu", c)] = sacc[ds(64 - 64 * h, HD), :]
                    nc.sync.dma_start(
                        out=st["den16"][ds(4 * c, 4), :],
                        in_=sacc[ds(64 * h, 1), :])

                def rec_piece():
                    # den16 rows are already q-ordered: den16[R, m] =
                    # den(q = R*128 + m). Reciprocal in this layout (16
                    # lanes), then one partition-major DMA linearizes to
                    # rec_row[0, q]. bf16 rec_row: fp32 matmuls run in
                    # LOW_HIGH double-pass mode (~1us for N=512).
                    den32 = stg.tile([16, 128], fp32, tag="den32")
                    nc.vector.tensor_copy(den32, st["den16"])
                    rec32 = stg.tile([16, 128], fp32, tag="rec32")
                    nc.vector.reciprocal_approx_fast(rec32, den32)
                    rec16 = stg.tile([16, 128], fp16, tag="rec16")
                    nc.vector.tensor_copy(rec16, rec32)
                    rr = stg.tile([1, S], fp16, tag="rr", bufs=2)
                    nc.sync.dma_start(
                        out=bass.AP(tensor=rr.tensor, offset=rr.offset,
                                    ap=[[rr.ap[0][0], 1], [1, S]]),
                        in_=rec16[:, :])
                    for c in range(NPV):
                        recb = pvps.tile([HD, QC], fp32, tag="pv")
                        nc.tensor.matmul(
                            recb, lhsT=ones64f_sb[0:1, :],
                            rhs=rr[0:1, ds(c * QC, QC)],
                            start=True, stop=True)
                        nc.vector.tensor_mul(
                            hT[b][ds(h * HD, HD), ds(c * QC, QC)],
                            st[("hu", c)], recb)
                    if KDEBUG and i == 0:
                        nc.sync.dma_start(out=dbg_rr[:, :], in_=rr)

                for c in range(NPV):
                    pieces.append(lambda c=c: mm_piece(c))
                pieces.append(rec_piece)
                return pieces

            # ---- phase 3: output projection (partial, transposed) ----
            def oproj_piece(b, ft, cc, tail=False):
                po = accps.tile([128, 512], fp32, tag="acc")
                nc.tensor.matmul(
                    po, lhsT=wo_sb[:, ts(ft, 128)],
                    rhs=hT[b][:, ds(cc * 512, 512)],
                    start=True, stop=True)
                ostage = work.tile([128, 512], bf16, tag="ostage")
                nc.vector.tensor_copy(ostage, po)
                nc.gpsimd.dma_start(
                    out=out_d[ft, b * (S // 512) + cc, :, :], in_=ostage)

            def oproj_pieces(b, fts, tail=False):
                def four(bb, f):
                    for c in range(S // 512):
                        oproj_piece(bb, f, c, tail=tail)
                return [lambda f=ft, bb=b: four(bb, f) for ft in fts]

            # ---- emission schedule ----
            def interleave(main, others, ratio=None):
                """Emit main pieces with `others` spliced evenly."""
                if ratio is None:
                    ratio = max(1, len(main) // max(1, len(others)))
                oi = 0
                for n, m in enumerate(main):
                    m()
                    if n % ratio == ratio - 1 and oi < len(others):
                        others[oi]()
                        oi += 1
                for o in others[oi:]:
                    o()

            # startup: batch-0 projection
            for p in proj_pieces(0):
                p()

            nprob = B * HPC
            pts = {}
            pvq = []      # pending P@V pieces (from previous problem)
            pending = []  # filler queue, carried across iterations
            for i in range(nprob + 1):
                # projection for batch i//2+1 split over iterations 2b-2,
                # 2b-1; O-proj for batch (i-3)//2 split over 2b+3, 2b+4.
                # Proj pieces go first: their casts feed the PE (rope
                # matmuls) soonest.
                bb = i // 2 + 1
                half = i % 2
                if bb < B:
                    pending += proj_pieces(bb)[10 * half:10 * half + 10]
                pending += pvq
                pvq = []
                if i >= 3:
                    ob = (i - 3) // 2
                    ohalf = (i - 3) % 2
                    pending += oproj_pieces(ob, range(4 * ohalf, 4 * ohalf + 4))
                if i == nprob:   # epilogue: last batch's O-proj
                    pending += oproj_pieces(B - 1, range(8), tail=True)
                if i < nprob:
                    pt = ptri_pool.tile([128, PTRI_W], bf16, tag="pt")
                    pts[i] = pt
                    wins = score_pieces(i, pt)
                    nwin = len(wins)
                    for wi, wpc in enumerate(wins):
                        wpc()
                        # drain fillers proportionally; leftovers carry
                        # into the next iteration instead of dumping here
                        want = (len(pending) + nwin - wi - 1) // (nwin - wi)
                        for _ in range(min(want, 2)):
                            if pending:
                                pending.pop(0)()
                    pvq = pv_pieces(i, pt)
                else:
                    for p in pending:
                        p()
                    pending = []
            # pv of the last problem ran inside the epilogue fillers
            if KDEBUG:
                for bb in range(B):
                    nc.sync.dma_start(out=dbg_q[:, ts(bb, S)],
                                      in_=get_qk(bb, "q"))
                    nc.sync.dma_start(out=dbg_k[:, ts(bb, S)],
                                      in_=get_qk(bb, "k"))
                    nc.sync.dma_start(out=dbg_h[:, ts(bb, S)], in_=hT[bb])

    nc.compile()
    return nc


_NC_CACHE = None


def _get_nc():
    global _NC_CACHE
    if _NC_CACHE is None:
        _NC_CACHE = _build_nc()
    return _NC_CACHE


def build_in_maps(x, positions, Wqkv, bqkv, Wo, bo):
    xT = x.reshape(T, D).T.astype(BF16)            # [D, T]
    # chunk-block layout [tci, p, kt*512]: contiguous 4KB runs per partition
    xblk = np.ascontiguousarray(
        xT.reshape(KT, 128, NTC, TC).transpose(2, 1, 0, 3).reshape(NTC, 128, KT * TC))
    # RoPE tables, layout matched to the stacked-heads partition dim:
    # partition p -> head-local pair i = (p % 64) // 2
    i_of_p = (np.arange(PC) % HD) // 2
    invf = 1.0 / (ROPE_THETA ** (2.0 * i_of_p / HD))          # [128]
    ang = invf[:, None] * np.asarray(positions[0])[None, :]   # [128, S]
    cost = np.ascontiguousarray(np.cos(ang)).astype(BF16)
    sint = np.ascontiguousarray(np.sin(ang)).astype(BF16)
    in_maps = []
    for c in range(NCORES):
        r0 = c * PC
        wq = np.ascontiguousarray(Wqkv[r0:r0 + PC, :].T).astype(BF16)
        wk = np.ascontiguousarray(Wqkv[D + r0:D + r0 + PC, :].T).astype(BF16)
        wv = np.ascontiguousarray(Wqkv[2 * D + r0:2 * D + r0 + PC, :].T).astype(BF16)
        wo = np.ascontiguousarray(Wo[:, r0:r0 + PC].T).astype(BF16)
        in_maps.append({
            "x": xblk, "cost": cost, "sint": sint,
            "wq": wq, "wk": wk, "wv": wv, "wo": wo,
            "bq": bqkv[r0:r0 + PC].astype(np.float32),
            "bk": bqkv[D + r0:D + r0 + PC].astype(np.float32),
            "bv": bqkv[2 * D + r0:2 * D + r0 + PC].astype(np.float32),
        })
    return in_maps


def assemble_out(res, bo):
    acc = res.results[0]["out"].astype(np.float32)
    for c in range(1, NCORES):
        acc += res.results[c]["out"].astype(np.float32)
    # [KT, T//512, 128, 512] -> [D, T]
    full = acc.transpose(0, 2, 1, 3).reshape(D, T)
    out = full + bo[:, None].astype(np.float32)
    return np.ascontiguousarray(out.T).reshape(B, S, D)


def kernel(x, positions, Wqkv, bqkv, Wo, bo):
    x = np.asarray(x)
    positions = np.asarray(positions)
    Wqkv = np.asarray(Wqkv)
    bqkv = np.asarray(bqkv)
    Wo = np.asarray(Wo)
    bo = np.asarray(bo)
    nc = _get_nc()
    in_maps = build_in_maps(x, positions, Wqkv, bqkv, Wo, bo)
    res = run_bass_kernel_spmd(nc, in_maps, core_ids=list(range(NCORES)))
    return assemble_out(res, bo)


# revision 28
# speedup vs baseline: 1.1702x; 1.1702x over previous
"""Multi-head self-attention (RoPE, causal) Trainium2 Bass kernel.

Problem: B=4, S=2048, D=1024, H=16 heads, hd=64, fused QKV + RoPE +
causal softmax attention + output projection (torch-Linear convention).

Sharding: Megatron-style tensor parallel over heads. Each of the 8
NeuronCores owns 2 heads: it projects the full token stream through its
128-row slices of Wq/Wk/Wv, applies RoPE, runs causal attention for its
2 heads x 4 batches, and computes a partial output projection
h_core @ Wo[:, core_slice].T  (transposed layout). The host sums the 8
partial outputs and adds the output bias.

v4 design notes (vs v2 baseline at ~601us; this version ~490us):
  - RoPE cos/sin tables are computed host-side in build_in_maps from the
    positions input (fp64 numpy -> bf16) and DMA'd in. This removed the
    on-device range-reduced Sin pipeline (22us of DVE at startup + 16
    activations) and cut the end-to-end relative error 1.8% -> 0.76%
    (the bf16 Sin spline was the dominant error source).
  - Score psum groups are 1024 wide (2 banks, double buffered = 4
    banks); each exp ACTIVATE covers 1024 columns, halving scalar-engine
    per-instruction overhead (136 calls).
  - Softmax denominator path: den rows gather to a q-ordered [16,128]
    fp16 tile, fp32 reciprocal_approx_fast in that layout, one
    partition-major DMA linearizes to rec_row[1,S] fp16, and a K=1
    ones-matmul broadcasts 1/den to [64,512] psum (rides the pv psum
    pool). This replaced the ejs/idf selector matmuls (128 MMs), a PE
    transpose, and two dedicated psum banks.
  - P@V staging: one [128,512] fp16 CAST per chunk captures h values
    AND den rows; hu/den are views into it.
  - Emission scheduling: proj chunk pieces are decoupled
    ([mm q][mm k][rope q][rope k][v]) so each psum group's DVE cast has
    ~2 pieces of slack before its consumer matmul; fillers (proj of the
    next batch, P@V of the previous problem, o-proj) live in a queue
    that drains proportionally between score windows and carries across
    iterations, keeping the PE fed so the HAM clock gate stays warm.
  - RoPE final add and the causal masks run on gpsimd; o-proj and P@V
    psum->sbuf casts run on DVE; the scalar engine runs only Exp.
  - qT/kT residents rotate through 2 buffers; x chunk 0 is DMA'd in 8
    per-ktile slices so the first projection matmul starts ~5us earlier.
"""

import os
import sys

for _p in ("/opt/trn_rl_repo",):
    if os.path.isdir(_p) and _p not in sys.path:
        sys.path.append(_p)

import math

import ml_dtypes
import numpy as np

import concourse.bass as bass
import concourse.mybir as mybir
import concourse.tile as tile
from concourse import bacc
from concourse.bass import ts, ds
from concourse.bass_utils import run_bass_kernel_spmd

BF16 = ml_dtypes.bfloat16

B = 4
S = 2048
D = 1024
H = 16
HD = 64
NCORES = 8
HPC = H // NCORES          # heads per core = 2
PC = HPC * HD              # partition rows per core's heads = 128
T = B * S                  # 8192 tokens
KT = D // 128              # f_in k-tiles = 8
NTOK = T // 128            # 64 token tiles of 128
SCALE = 1.0 / math.sqrt(HD)
ROPE_THETA = 10000.0

TWO_PI = 2.0 * math.pi
INV_2PI = 1.0 / TWO_PI
MAGIC = 12582912.0         # 1.5 * 2**23, float32 round-to-nearest trick
HALF_PI = math.pi / 2.0

NQT = S // 128             # 16 q/k tiles per sequence
# triangular packing offsets for P_T: row kt covers q in [kt*128, S)
OFFS = [0] * NQT
for _kt in range(1, NQT):
    OFFS[_kt] = OFFS[_kt - 1] + (S - (_kt - 1) * 128)
PTRI_W = OFFS[-1] + (S - (NQT - 1) * 128)   # 17408

TC = 512                   # token chunk for projections
NTC = T // TC              # 16
CPB = S // TC              # proj chunks per batch = 4
QC = 512                   # P@V q-chunk width
NPV = S // QC              # P@V chunks per problem = 4
SGW = 1024                 # scores psum group width (2 banks)


def _build_nc():
    nc = bacc.Bacc("TRN2", target_bir_lowering=False, debug=False,
                   num_devices=NCORES)
    dt = mybir.dt

    # ---- I/O ----
    x_in = nc.dram_tensor("x", [NTC, 128, KT * TC], dt.bfloat16,
                          kind="ExternalInput")
    cos_in = nc.dram_tensor("cost", [128, S], dt.bfloat16, kind="ExternalInput")
    sin_in = nc.dram_tensor("sint", [128, S], dt.bfloat16, kind="ExternalInput")
    wq_in = nc.dram_tensor("wq", [D, PC], dt.bfloat16, kind="ExternalInput")
    wk_in = nc.dram_tensor("wk", [D, PC], dt.bfloat16, kind="ExternalInput")
    wv_in = nc.dram_tensor("wv", [D, PC], dt.bfloat16, kind="ExternalInput")
    wo_in = nc.dram_tensor("wo", [PC, D], dt.bfloat16, kind="ExternalInput")
    bq_in = nc.dram_tensor("bq", [PC], dt.float32, kind="ExternalInput")
    bk_in = nc.dram_tensor("bk", [PC], dt.float32, kind="ExternalInput")
    bv_in = nc.dram_tensor("bv", [PC], dt.float32, kind="ExternalInput")
    out_d = nc.dram_tensor("out", [KT, T // 512, 128, 512], dt.bfloat16,
                           kind="ExternalOutput")
    KDEBUG = os.environ.get("KDEBUG") == "1"
    if KDEBUG:
        dbg_q = nc.dram_tensor("dbg_q", [128, T], dt.bfloat16, kind="ExternalOutput")
        dbg_k = nc.dram_tensor("dbg_k", [128, T], dt.bfloat16, kind="ExternalOutput")
        dbg_h = nc.dram_tensor("dbg_h", [128, T], dt.bfloat16, kind="ExternalOutput")
        dbg_pt = nc.dram_tensor("dbg_pt", [128, PTRI_W], dt.bfloat16,
                                kind="ExternalOutput")
        dbg_rr = nc.dram_tensor("dbg_rr", [1, S], dt.float16,
                                kind="ExternalOutput")

    # ---- inline constants ----
    # RT = R.T where (R @ q)[2i] = -q[2i+1], (R @ q)[2i+1] = q[2i],
    # block-diagonal over the 2 stacked heads (structure identical).
    r = np.zeros((PC, PC), dtype=np.float32)
    for h in range(HPC):
        for i in range(HD // 2):
            r[h * HD + 2 * i, h * HD + 2 * i + 1] = -1.0
            r[h * HD + 2 * i + 1, h * HD + 2 * i] = 1.0
    rt_np = np.ascontiguousarray(r.T).astype(BF16)
    # causal mask for diagonal scoresT blocks: keep k_local <= q_local
    mask_np = np.tril(np.ones((128, 128), dtype=np.float32)).T.astype(BF16)
    rt_d = nc.inline_tensor(rt_np, "rt_c")
    ones64_np = np.ones((1, HD), dtype=np.float16)
    ones64_d = nc.inline_tensor(ones64_np, "ones64_c")
    mask_d = nc.inline_tensor(mask_np, "mask_c")

    fp32 = dt.float32
    bf16 = dt.bfloat16
    fp16 = dt.float16

    with tile.TileContext(nc) as tc:
        with (
            tc.tile_pool(name="consts", bufs=1) as consts,
            tc.tile_pool(name="resid", bufs=1) as resid,
            tc.tile_pool(name="xp", bufs=2) as xp,
            tc.tile_pool(name="work", bufs=2) as work,
            tc.tile_pool(name="vst", bufs=2) as vst,
            tc.tile_pool(name="stg", bufs=2) as stg,
            tc.tile_pool(name="ptri", bufs=2) as ptri_pool,
            tc.tile_pool(name="bigps", bufs=2, space="PSUM") as bigps,
            tc.tile_pool(name="pvps", bufs=2, space="PSUM") as pvps,
            tc.tile_pool(name="accps", bufs=2, space="PSUM") as accps,
        ):
            # ---- load constants / weights to SBUF ----
            # x chunk 0 DMA first so the PE can start ASAP; weights on
            # separate queues so wq doesn't queue behind everything.
            xt0 = xp.tile([128, KT, TC], bf16, tag="xt")
            for _kt in range(KT):
                nc.scalar.dma_start(out=xt0[:, _kt, :],
                                    in_=x_in[0, :, ds(_kt * TC, TC)])

            wq_sb = consts.tile([128, KT, PC], bf16, tag="wq")
            wk_sb = consts.tile([128, KT, PC], bf16, tag="wk")
            wv_sb = consts.tile([128, KT, PC], bf16, tag="wv")
            nc.sync.dma_start(
                out=wq_sb, in_=wq_in.ap().rearrange("(kt p) m -> p kt m", p=128))
            nc.gpsimd.dma_start(
                out=wk_sb, in_=wk_in.ap().rearrange("(kt p) m -> p kt m", p=128))
            nc.scalar.dma_start(
                out=wv_sb, in_=wv_in.ap().rearrange("(kt p) m -> p kt m", p=128))
            wo_sb = consts.tile([128, D], bf16, tag="wo")
            nc.gpsimd.dma_start(out=wo_sb, in_=wo_in[:, :])
            rt_sb = consts.tile([128, 128], bf16, tag="rt")
            nc.sync.dma_start(out=rt_sb, in_=rt_d[:, :])
            mask_sb = consts.tile([128, 128], bf16, tag="mask")
            nc.sync.dma_start(out=mask_sb, in_=mask_d[:, :])
            ones64f_sb = consts.tile([1, HD], fp16, tag="ones64")
            nc.sync.dma_start(out=ones64f_sb, in_=ones64_d[:, :])
            bq_sb = consts.tile([128, 1], fp32, tag="bq")
            nc.sync.dma_start(out=bq_sb, in_=bq_in.ap().rearrange("(p o) -> p o", o=1))
            bk_sb = consts.tile([128, 1], fp32, tag="bk")
            nc.sync.dma_start(out=bk_sb, in_=bk_in.ap().rearrange("(p o) -> p o", o=1))
            bv_sb = consts.tile([128, 1], fp32, tag="bv")
            nc.sync.dma_start(out=bv_sb, in_=bv_in.ap().rearrange("(p o) -> p o", o=1))

            # ---- residents ----
            # qT/kT rotate through 2 buffers (proj of batch b+1 overlaps
            # scores of batch b); hT stays per-batch (read by o-proj two
            # problems later).
            qkh = {}

            def get_qk(bb, which):
                if (bb, which) not in qkh:
                    t = resid.tile([128, S], bf16, tag=which, bufs=2,
                                   name=f"{which}{bb}")
                    qkh[(bb, which)] = t
                return qkh[(bb, which)]

            hT = []
            for bb in range(B):
                th = resid.tile([128, S], bf16, tag=f"hT{bb}")
                hT.append(th)
            # v natural as repeating [ones(64) | d_h0(64) | d_h1(64)]
            # 192-col blocks (plus one trailing ones block): head0's P@V
            # lhsT is [ones|d0] (den in psum rows 0-63, h in 64-127) and
            # head1's is [d1|ones-of-next-block] (h in 0-63, den 64-127) --
            # both plain contiguous 128-col slices.
            NTB = NTOK // B            # 16 tok tiles per batch
            VW = NTB * 192 + 64
            vN = []
            for bb in range(B):
                tv = resid.tile([128, VW], bf16, tag=f"vN{bb}")
                vN.append(tv)
                nc.vector.memset(
                    bass.AP(tensor=tv.tensor, offset=tv.offset,
                            ap=[tv.ap[0], [192, NTB + 1], [1, 64]]), 1.0)
            # RoPE cos/sin tables [128, S] bf16, computed host-side
            cos_sb = consts.tile([128, S], bf16, tag="cosT")
            nc.gpsimd.dma_start(out=cos_sb, in_=cos_in[:, :])
            sin_sb = consts.tile([128, S], bf16, tag="sinT")
            nc.gpsimd.dma_start(out=sin_sb, in_=sin_in[:, :])

            # ---- phase 1: QKV projection + RoPE, per token chunk ----
            xt_cache = {0: xt0}

            def get_xt(tci):
                if tci not in xt_cache:
                    xt = xp.tile([128, KT, TC], bf16, tag="xt")
                    nc.scalar.dma_start(out=xt.rearrange("p a b -> p (a b)"),
                                        in_=x_in[tci, :, :])
                    xt_cache[tci] = xt
                return xt_cache[tci]

            def proj_qk_mm(tci, which, st):
                """q/k projection matmuls for one 512-token chunk."""
                xt = get_xt(tci)
                if tci + 1 < NTC:
                    get_xt(tci + 1)
                w_sb, b_sb = ((wq_sb, bq_sb) if which == "q"
                              else (wk_sb, bk_sb))
                pa = accps.tile([128, TC], fp32, tag="acc")
                for kt in range(KT):
                    nc.tensor.matmul(pa, lhsT=w_sb[:, kt, :],
                                     rhs=xt[:, kt, :],
                                     start=(kt == 0), stop=(kt == KT - 1))
                a_sb = work.tile([128, TC], bf16, tag="a_sb")
                nc.vector.tensor_scalar_add(a_sb, pa, b_sb)
                st[which] = a_sb

            def proj_qk_rope(tci, which, st):
                """RoPE for the chunk: rotation matmul + combine."""
                dest = get_qk(tci // CPB, which)
                a_sb = st.pop(which)
                pb = accps.tile([128, TC], fp32, tag="acc")
                nc.tensor.matmul(pb, lhsT=rt_sb, rhs=a_sb,
                                 start=True, stop=True)
                ssl = ds((tci * TC) % S, TC)
                t1 = work.tile([128, TC], bf16, tag="t1")
                nc.vector.tensor_mul(t1, a_sb, cos_sb[:, ssl])
                t2 = work.tile([128, TC], bf16, tag="t2")
                nc.vector.tensor_mul(t2, pb, sin_sb[:, ssl])
                nc.gpsimd.tensor_add(dest[:, ts(tci % CPB, TC)], t1, t2)

            def proj_v(tci):
                """v projection, transposed production (wv stationary),
                then DMA-transposed into natural layout vN on sync."""
                xt = get_xt(tci)
                pv = accps.tile([128, TC], fp32, tag="acc")
                for kt in range(KT):
                    nc.tensor.matmul(pv, lhsT=wv_sb[:, kt, :],
                                     rhs=xt[:, kt, :],
                                     start=(kt == 0), stop=(kt == KT - 1))
                vTst = vst.tile([128, TC], bf16, tag="vTst")
                nc.vector.tensor_scalar_add(vTst, pv, bv_sb)
                for sub in range(TC // 128):
                    tl = (tci % CPB) * (TC // 128) + sub
                    nc.sync.dma_start_transpose(
                        vN[tci // CPB][:, ds(192 * tl + 64, 128)],
                        vTst[:, ts(sub, 128)])

            def proj_pieces(b):
                out = []
                for cc in range(CPB):
                    tci = b * CPB + cc
                    st = {}
                    out.append(lambda t=tci, s=st: proj_qk_mm(t, "q", s))
                    out.append(lambda t=tci, s=st: proj_qk_mm(t, "k", s))
                    out.append(lambda t=tci, s=st: proj_qk_rope(t, "q", s))
                    out.append(lambda t=tci, s=st: proj_qk_rope(t, "k", s))
                    out.append(lambda t=tci: proj_v(t))
                return out

            # ---- phase 2a: scores + exp + mask for one (batch, head) ----
            def score_pieces(i, pt):
                """Closures, each = one <=1024-col psum group: matmul the
                row pieces intersecting the window, exp into pt, mask
                any diagonal blocks fully inside the window."""
                b, h = i // HPC, i % HPC
                hsl = ds(h * HD, HD)
                kTb = get_qk(b, "k")
                qTb = get_qk(b, "q")
                pieces = []
                x0 = 0
                while x0 < PTRI_W:
                    w = min(SGW, PTRI_W - x0)

                    def piece(x0=x0, w=w):
                        sc = bigps.tile([128, SGW], fp32, tag="big")
                        # rows intersecting flat-span window [x0, x0+w)
                        for kt in range(NQT):
                            r0, r1 = OFFS[kt], OFFS[kt] + (S - kt * 128)
                            lo, hi = max(r0, x0), min(r1, x0 + w)
                            if lo >= hi:
                                continue
                            q0 = kt * 128 + (lo - r0)
                            # split at 512-aligned psum columns: a matmul
                            # output must not cross a PSUM bank boundary
                            cuts = [lo]
                            nb = (lo - x0) // 512 * 512 + 512 + x0
                            while nb < hi:
                                cuts.append(nb)
                                nb += 512
                            cuts.append(hi)
                            for aa, bb in zip(cuts, cuts[1:]):
                                nc.tensor.matmul(
                                    sc[:, ds(aa - x0, bb - aa)],
                                    lhsT=kTb[hsl, ds(kt * 128, 128)],
                                    rhs=qTb[hsl, ds(q0 + (aa - lo), bb - aa)],
                                    start=True, stop=True)
                        nc.scalar.activation(
                            pt[:, ds(x0, w)], sc[:, 0:w],
                            mybir.ActivationFunctionType.Exp, scale=SCALE)
                        for kt in range(NQT):
                            if x0 <= OFFS[kt] and OFFS[kt] + 128 <= x0 + w:
                                dsl = ds(OFFS[kt], 128)
                                nc.gpsimd.tensor_mul(
                                    pt[:, dsl], pt[:, dsl], mask_sb)

                    pieces.append(piece)
                    x0 += w
                return pieces

            # ---- phase 2b: P@V (V stationary) + normalize into hT ----
            # Each P@V chunk's psum acc [128,512] (den rows + h rows) is
            # cast once to fp16 staging; den rows gather into [16,128],
            # DMA-transpose to [128,16], DVE reciprocal -> fp16, one DMA
            # scatters to rec_row [1,S] (q-order), and a K=1 ones-matmul
            # broadcasts 1/den to [64,512] psum for the normalize mult.
            def pv_pieces(i, pt):
                b, h = i // HPC, i % HPC
                st = {}
                pieces = []

                def mm_piece(c):
                    q0 = c * QC
                    acc = pvps.tile([128, QC], fp32, tag="pv")
                    kts = [kt for kt in range(NQT) if kt * 128 < q0 + QC]
                    for j, kt in enumerate(kts):
                        lo = max(kt * 128, q0)
                        w = q0 + QC - lo
                        lt = vN[b][:, ds(192 * kt + 128 * h, 128)]
                        nc.tensor.matmul(
                            acc[:, ds(lo - q0, w)], lhsT=lt,
                            rhs=pt[:, ds(OFFS[kt] + lo - kt * 128, w)],
                            start=(j == 0), stop=(j == len(kts) - 1))
                    if c == 0:
                        d16 = stg.tile([16, 128], fp16, tag="den16")
                        st["den16"] = d16
                    sacc = stg.tile([128, QC], fp16, tag="sacc", bufs=5)
                    nc.vector.tensor_copy(sacc, acc)
                    st[("hu", c)] = sacc[ds(64 - 64 * h, HD), :]
                    nc.sync.dma_start(
                        out=st["den16"][ds(4 * c, 4), :],
                        in_=sacc[ds(64 * h, 1), :])

                def rec_piece():
                    # den16 rows are already q-ordered: den16[R, m] =
                    # den(q = R*128 + m). Reciprocal in this layout (16
                    # lanes), then one partition-major DMA linearizes to
                    # rec_row[0, q]. bf16 rec_row: fp32 matmuls run in
                    # LOW_HIGH double-pass mode (~1us for N=512).
                    den32 = stg.tile([16, 128], fp32, tag="den32")
                    nc.vector.tensor_copy(den32, st["den16"])
                    rec32 = stg.tile([16, 128], fp32, tag="rec32")
                    nc.vector.reciprocal_approx_fast(rec32, den32)
                    rec16 = stg.tile([16, 128], fp16, tag="rec16")
                    nc.vector.tensor_copy(rec16, rec32)
                    rr = stg.tile([1, S], fp16, tag="rr", bufs=2)
                    nc.sync.dma_start(
                        out=bass.AP(tensor=rr.tensor, offset=rr.offset,
                                    ap=[[rr.ap[0][0], 1], [1, S]]),
                        in_=rec16[:, :])
                    for c in range(NPV):
                        recb = pvps.tile([HD, QC], fp32, tag="pv")
                        nc.tensor.matmul(
                            recb, lhsT=ones64f_sb[0:1, :],
                            rhs=rr[0:1, ds(c * QC, QC)],
                            start=True, stop=True)
                        nc.vector.tensor_mul(
                            hT[b][ds(h * HD, HD), ds(c * QC, QC)],
                            st[("hu", c)], recb)
                    if KDEBUG and i == 0:
                        nc.sync.dma_start(out=dbg_rr[:, :], in_=rr)

                for c in range(NPV):
                    pieces.append(lambda c=c: mm_piece(c))
                pieces.append(rec_piece)
                return pieces

            # ---- phase 3: output projection (partial, transposed) ----
            def oproj_piece(b, ft, cc, tail=False):
                po = accps.tile([128, 512], fp32, tag="acc")
                nc.tensor.matmul(
                    po, lhsT=wo_sb[:, ts(ft, 128)],
                    rhs=hT[b][:, ds(cc * 512, 512)],
                    start=True, stop=True)
                ostage = work.tile([128, 512], bf16, tag="ostage")
                nc.vector.tensor_copy(ostage, po)
                nc.gpsimd.dma_start(
                    out=out_d[ft, b * (S // 512) + cc, :, :], in_=ostage)

            def oproj_pieces(b, fts, tail=False):
                def four(bb, f):
                    for c in range(S // 512):
                        oproj_piece(bb, f, c, tail=tail)
                return [lambda f=ft, bb=b: four(bb, f) for ft in fts]

            # ---- emission schedule ----
            def interleave(main, others, ratio=None):
                """Emit main pieces with `others` spliced evenly."""
                if ratio is None:
                    ratio = max(1, len(main) // max(1, len(others)))
                oi = 0
                for n, m in enumerate(main):
                    m()
                    if n % ratio == ratio - 1 and oi < len(others):
                        others[oi]()
                        oi += 1
                for o in others[oi:]:
                    o()

            # startup: batch-0 projection
            for p in proj_pieces(0):
                p()

            nprob = B * HPC
            pts = {}
            pvq = []      # pending P@V pieces (from previous problem)
            pending = []  # filler queue, carried across iterations
            for i in range(nprob + 1):
                # projection for batch i//2+1 split over iterations 2b-2,
                # 2b-1; O-proj for batch (i-3)//2 split over 2b+3, 2b+4.
                # Proj pieces go first: their casts feed the PE (rope
                # matmuls) soonest.
                bb = i // 2 + 1
                half = i % 2
                if bb < B:
                    pending += proj_pieces(bb)[10 * half:10 * half + 10]
                pending += pvq
                pvq = []
                if i >= 3:
                    ob = (i - 3) // 2
                    ohalf = (i - 3) % 2
                    pending += oproj_pieces(ob, range(4 * ohalf, 4 * ohalf + 4))
                if i == nprob:   # epilogue: last batch's O-proj
                    pending += oproj_pieces(B - 1, range(8), tail=True)
                if i < nprob:
                    pt = ptri_pool.tile([128, PTRI_W], bf16, tag="pt")
                    pts[i] = pt
                    wins = score_pieces(i, pt)
                    nwin = len(wins)
                    for wi, wpc in enumerate(wins):
                        wpc()
                        # drain fillers proportionally; leftovers carry
                        # into the next iteration instead of dumping here
                        want = (len(pending) + nwin - wi - 1) // (nwin - wi)
                        for _ in range(min(want, 2)):
                            if pending:
                                pending.pop(0)()
                    pvq = pv_pieces(i, pt)
                else:
                    for p in pending:
                        p()
                    pending = []
            # pv of the last problem ran inside the epilogue fillers
            if KDEBUG:
                for bb in range(B):
                    nc.sync.dma_start(out=dbg_q[:, ts(bb, S)],
                                      in_=get_qk(bb, "q"))
                    nc.sync.dma_start(out=dbg_k[:, ts(bb, S)],
                                      in_=get_qk(bb, "k"))
                    nc.sync.dma_start(out=dbg_h[:, ts(bb, S)], in_=hT[bb])

    nc.compile()
    return nc


_NC_CACHE = None


def _get_nc():
    global _NC_CACHE
    if _NC_CACHE is None:
        _NC_CACHE = _build_nc()
    return _NC_CACHE


def build_in_maps(x, positions, Wqkv, bqkv, Wo, bo):
    xT = x.reshape(T, D).T.astype(BF16)            # [D, T]
    # chunk-block layout [tci, p, kt*512]: contiguous 4KB runs per partition
    xblk = np.ascontiguousarray(
        xT.reshape(KT, 128, NTC, TC).transpose(2, 1, 0, 3).reshape(NTC, 128, KT * TC))
    # RoPE tables, layout matched to the stacked-heads partition dim:
    # partition p -> head-local pair i = (p % 64) // 2
    i_of_p = (np.arange(PC) % HD) // 2
    invf = 1.0 / (ROPE_THETA ** (2.0 * i_of_p / HD))          # [128]
    ang = invf[:, None] * np.asarray(positions[0])[None, :]   # [128, S]
    cost = np.ascontiguousarray(np.cos(ang)).astype(BF16)
    sint = np.ascontiguousarray(np.sin(ang)).astype(BF16)
    in_maps = []
    for c in range(NCORES):
        r0 = c * PC
        wq = np.ascontiguousarray(Wqkv[r0:r0 + PC, :].T).astype(BF16)
        wk = np.ascontiguousarray(Wqkv[D + r0:D + r0 + PC, :].T).astype(BF16)
        wv = np.ascontiguousarray(Wqkv[2 * D + r0:2 * D + r0 + PC, :].T).astype(BF16)
        wo = np.ascontiguousarray(Wo[:, r0:r0 + PC].T).astype(BF16)
        in_maps.append({
            "x": xblk, "cost": cost, "sint": sint,
            "wq": wq, "wk": wk, "wv": wv, "wo": wo,
            "bq": bqkv[r0:r0 + PC].astype(np.float32),
            "bk": bqkv[D + r0:D + r0 + PC].astype(np.float32),
            "bv": bqkv[2 * D + r0:2 * D + r0 + PC].astype(np.float32),
        })
    return in_maps


def assemble_out(res, bo):
    acc = res.results[0]["out"].astype(np.float32)
    for c in range(1, NCORES):
        acc += res.results[c]["out"].astype(np.float32)
    # [KT, T//512, 128, 512] -> [D, T]
    full = acc.transpose(0, 2, 1, 3).reshape(D, T)
    out = full + bo[:, None].astype(np.float32)
    return np.ascontiguousarray(out.T).reshape(B, S, D)


def kernel(x, positions, Wqkv, bqkv, Wo, bo):
    x = np.asarray(x)
    positions = np.asarray(positions)
    Wqkv = np.asarray(Wqkv)
    bqkv = np.asarray(bqkv)
    Wo = np.asarray(Wo)
    bo = np.asarray(bo)
    nc = _get_nc()
    in_maps = build_in_maps(x, positions, Wqkv, bqkv, Wo, bo)
    res = run_bass_kernel_spmd(nc, in_maps, core_ids=list(range(NCORES)))
    return assemble_out(res, bo)


# revision 29
# speedup vs baseline: 1.3227x; 1.1304x over previous
"""Multi-head self-attention (RoPE, causal) Trainium2 Bass kernel.

Problem: B=4, S=2048, D=1024, H=16 heads, hd=64, fused QKV + RoPE +
causal softmax attention + output projection (torch-Linear convention).

Sharding: Megatron-style tensor parallel over heads. Each of the 8
NeuronCores owns 2 heads: it projects the full token stream through its
128-row slices of Wq/Wk/Wv, applies RoPE, runs causal attention for its
2 heads x 4 batches, and computes a partial output projection
h_core @ Wo[:, core_slice].T  (transposed layout). The host sums the 8
partial outputs and adds the output bias.

v4 design notes (vs v2 baseline at ~601us; this version ~490us):
  - RoPE cos/sin tables are computed host-side in build_in_maps from the
    positions input (fp64 numpy -> bf16) and DMA'd in. This removed the
    on-device range-reduced Sin pipeline (22us of DVE at startup + 16
    activations) and cut the end-to-end relative error 1.8% -> 0.76%
    (the bf16 Sin spline was the dominant error source).
  - Score psum groups are 1024 wide (2 banks, double buffered = 4
    banks); each exp ACTIVATE covers 1024 columns, halving scalar-engine
    per-instruction overhead (136 calls).
  - Softmax denominator path: den rows gather to a q-ordered [16,128]
    fp16 tile, fp32 reciprocal_approx_fast in that layout, one
    partition-major DMA linearizes to rec_row[1,S] fp16, and a K=1
    ones-matmul broadcasts 1/den to [64,512] psum (rides the pv psum
    pool). This replaced the ejs/idf selector matmuls (128 MMs), a PE
    transpose, and two dedicated psum banks.
  - P@V staging: one [128,512] fp16 CAST per chunk captures h values
    AND den rows; hu/den are views into it.
  - Emission scheduling: proj chunk pieces are decoupled
    ([mm q][mm k][rope q][rope k][v]) so each psum group's DVE cast has
    ~2 pieces of slack before its consumer matmul; fillers (proj of the
    next batch, P@V of the previous problem, o-proj) live in a queue
    that drains proportionally between score windows and carries across
    iterations, keeping the PE fed so the HAM clock gate stays warm.
  - RoPE final add and the causal masks run on gpsimd; o-proj and P@V
    psum->sbuf casts run on DVE; the scalar engine runs only Exp.
  - qT/kT residents rotate through 2 buffers; x chunk 0 is DMA'd in 8
    per-ktile slices so the first projection matmul starts ~5us earlier.
"""

import os
import sys

for _p in ("/opt/trn_rl_repo",):
    if os.path.isdir(_p) and _p not in sys.path:
        sys.path.append(_p)

import math

import ml_dtypes
import numpy as np

import concourse.bass as bass
import concourse.mybir as mybir
import concourse.tile as tile
from concourse import bacc
from concourse.bass import ts, ds
from concourse.bass_utils import run_bass_kernel_spmd

BF16 = ml_dtypes.bfloat16

B = 4
S = 2048
D = 1024
H = 16
HD = 64
NCORES = 8
HPC = H // NCORES          # heads per core = 2
PC = HPC * HD              # partition rows per core's heads = 128
T = B * S                  # 8192 tokens
KT = D // 128              # f_in k-tiles = 8
NTOK = T // 128            # 64 token tiles of 128
SCALE = 1.0 / math.sqrt(HD)
ROPE_THETA = 10000.0

TWO_PI = 2.0 * math.pi
INV_2PI = 1.0 / TWO_PI
MAGIC = 12582912.0         # 1.5 * 2**23, float32 round-to-nearest trick
HALF_PI = math.pi / 2.0

NQT = S // 128             # 16 q/k tiles per sequence
# triangular packing offsets for P_T: row kt covers q in [kt*128, S)
OFFS = [0] * NQT
for _kt in range(1, NQT):
    OFFS[_kt] = OFFS[_kt - 1] + (S - (_kt - 1) * 128)
PTRI_W = OFFS[-1] + (S - (NQT - 1) * 128)   # 17408

TC = 512                   # token chunk for projections
NTC = T // TC              # 16
CPB = S // TC              # proj chunks per batch = 4
QC = 512                   # P@V q-chunk width
NPV = S // QC              # P@V chunks per problem = 4
SGW = 1024                 # scores psum group width (2 banks)


def _build_nc():
    nc = bacc.Bacc("TRN2", target_bir_lowering=False, debug=False,
                   num_devices=NCORES)
    dt = mybir.dt

    # ---- I/O ----
    x_in = nc.dram_tensor("x", [NTC, 128, KT * TC], dt.bfloat16,
                          kind="ExternalInput")
    cos_in = nc.dram_tensor("cost", [128, S], dt.bfloat16, kind="ExternalInput")
    sin_in = nc.dram_tensor("sint", [128, S], dt.bfloat16, kind="ExternalInput")
    wq_in = nc.dram_tensor("wq", [D, PC], dt.bfloat16, kind="ExternalInput")
    wk_in = nc.dram_tensor("wk", [D, PC], dt.bfloat16, kind="ExternalInput")
    wv_in = nc.dram_tensor("wv", [D, PC], dt.bfloat16, kind="ExternalInput")
    wo_in = nc.dram_tensor("wo", [PC, D], dt.bfloat16, kind="ExternalInput")
    bq_in = nc.dram_tensor("bq", [PC], dt.float32, kind="ExternalInput")
    bk_in = nc.dram_tensor("bk", [PC], dt.float32, kind="ExternalInput")
    bv_in = nc.dram_tensor("bv", [PC], dt.float32, kind="ExternalInput")
    out_d = nc.dram_tensor("out", [KT, T // 512, 128, 512], dt.bfloat16,
                           kind="ExternalOutput")
    KDEBUG = os.environ.get("KDEBUG") == "1"
    if KDEBUG:
        dbg_q = nc.dram_tensor("dbg_q", [128, T], dt.bfloat16, kind="ExternalOutput")
        dbg_k = nc.dram_tensor("dbg_k", [128, T], dt.bfloat16, kind="ExternalOutput")
        dbg_h = nc.dram_tensor("dbg_h", [128, T], dt.bfloat16, kind="ExternalOutput")
        dbg_pt = nc.dram_tensor("dbg_pt", [128, PTRI_W], dt.bfloat16,
                                kind="ExternalOutput")
        dbg_rr = nc.dram_tensor("dbg_rr", [1, S], dt.float16,
                                kind="ExternalOutput")

    # ---- inline constants ----
    # RT = R.T where (R @ q)[2i] = -q[2i+1], (R @ q)[2i+1] = q[2i],
    # block-diagonal over the 2 stacked heads (structure identical).
    r = np.zeros((PC, PC), dtype=np.float32)
    for h in range(HPC):
        for i in range(HD // 2):
            r[h * HD + 2 * i, h * HD + 2 * i + 1] = -1.0
            r[h * HD + 2 * i + 1, h * HD + 2 * i] = 1.0
    rt_np = np.ascontiguousarray(r.T).astype(BF16)
    # causal mask for diagonal scoresT blocks: keep k_local <= q_local
    mask_np = np.tril(np.ones((128, 128), dtype=np.float32)).T.astype(BF16)
    rt_d = nc.inline_tensor(rt_np, "rt_c")
    ones64_np = np.ones((1, HD), dtype=np.float16)
    ones64_d = nc.inline_tensor(ones64_np, "ones64_c")
    mask_d = nc.inline_tensor(mask_np, "mask_c")

    fp32 = dt.float32
    bf16 = dt.bfloat16
    fp16 = dt.float16

    with tile.TileContext(nc) as tc:
        with (
            tc.tile_pool(name="consts", bufs=1) as consts,
            tc.tile_pool(name="resid", bufs=1) as resid,
            tc.tile_pool(name="xp", bufs=2) as xp,
            tc.tile_pool(name="work", bufs=2) as work,
            tc.tile_pool(name="vst", bufs=2) as vst,
            tc.tile_pool(name="stg", bufs=2) as stg,
            tc.tile_pool(name="ptri", bufs=2) as ptri_pool,
            tc.tile_pool(name="bigps", bufs=2, space="PSUM") as bigps,
            tc.tile_pool(name="pvps", bufs=2, space="PSUM") as pvps,
            tc.tile_pool(name="accps", bufs=2, space="PSUM") as accps,
        ):
            # ---- load constants / weights to SBUF ----
            # x chunk 0 DMA first so the PE can start ASAP; weights on
            # separate queues so wq doesn't queue behind everything.
            xt0 = xp.tile([128, KT, TC], bf16, tag="xt")
            for _kt in range(KT):
                nc.scalar.dma_start(out=xt0[:, _kt, :],
                                    in_=x_in[0, :, ds(_kt * TC, TC)])

            wq_sb = consts.tile([128, KT, PC], bf16, tag="wq")
            wk_sb = consts.tile([128, KT, PC], bf16, tag="wk")
            wv_sb = consts.tile([128, KT, PC], bf16, tag="wv")
            nc.sync.dma_start(
                out=wq_sb, in_=wq_in.ap().rearrange("(kt p) m -> p kt m", p=128))
            nc.gpsimd.dma_start(
                out=wk_sb, in_=wk_in.ap().rearrange("(kt p) m -> p kt m", p=128))
            nc.scalar.dma_start(
                out=wv_sb, in_=wv_in.ap().rearrange("(kt p) m -> p kt m", p=128))
            wo_sb = consts.tile([128, D], bf16, tag="wo")
            nc.gpsimd.dma_start(out=wo_sb, in_=wo_in[:, :])
            rt_sb = consts.tile([128, 128], bf16, tag="rt")
            nc.sync.dma_start(out=rt_sb, in_=rt_d[:, :])
            mask_sb = consts.tile([128, 128], bf16, tag="mask")
            nc.sync.dma_start(out=mask_sb, in_=mask_d[:, :])
            ones64f_sb = consts.tile([1, HD], fp16, tag="ones64")
            nc.sync.dma_start(out=ones64f_sb, in_=ones64_d[:, :])
            bq_sb = consts.tile([128, 1], fp32, tag="bq")
            nc.sync.dma_start(out=bq_sb, in_=bq_in.ap().rearrange("(p o) -> p o", o=1))
            bk_sb = consts.tile([128, 1], fp32, tag="bk")
            nc.sync.dma_start(out=bk_sb, in_=bk_in.ap().rearrange("(p o) -> p o", o=1))
            bv_sb = consts.tile([128, 1], fp32, tag="bv")
            nc.sync.dma_start(out=bv_sb, in_=bv_in.ap().rearrange("(p o) -> p o", o=1))

            # ---- residents ----
            # qT/kT rotate through 2 buffers (proj of batch b+1 overlaps
            # scores of batch b); hT stays per-batch (read by o-proj two
            # problems later).
            qkh = {}

            def get_qk(bb, which):
                if (bb, which) not in qkh:
                    t = resid.tile([128, S], bf16, tag=which, bufs=2,
                                   name=f"{which}{bb}")
                    qkh[(bb, which)] = t
                return qkh[(bb, which)]

            hT = []
            for bb in range(B):
                th = resid.tile([128, S], bf16, tag=f"hT{bb}")
                hT.append(th)
            # v natural as repeating [ones(64) | d_h0(64) | d_h1(64)]
            # 192-col blocks (plus one trailing ones block): head0's P@V
            # lhsT is [ones|d0] (den in psum rows 0-63, h in 64-127) and
            # head1's is [d1|ones-of-next-block] (h in 0-63, den 64-127) --
            # both plain contiguous 128-col slices.
            NTB = NTOK // B            # 16 tok tiles per batch
            VW = NTB * 192 + 64
            vN = []
            for bb in range(B):
                tv = resid.tile([128, VW], bf16, tag=f"vN{bb}")
                vN.append(tv)
                nc.vector.memset(
                    bass.AP(tensor=tv.tensor, offset=tv.offset,
                            ap=[tv.ap[0], [192, NTB + 1], [1, 64]]), 1.0)
            # RoPE cos/sin tables [128, S] bf16, computed host-side
            cos_sb = consts.tile([128, S], bf16, tag="cosT")
            nc.gpsimd.dma_start(out=cos_sb, in_=cos_in[:, :])
            sin_sb = consts.tile([128, S], bf16, tag="sinT")
            nc.gpsimd.dma_start(out=sin_sb, in_=sin_in[:, :])

            # ---- phase 1: QKV projection + RoPE, per token chunk ----
            xt_cache = {0: xt0}

            def get_xt(tci):
                if tci not in xt_cache:
                    xt = xp.tile([128, KT, TC], bf16, tag="xt")
                    nc.scalar.dma_start(out=xt.rearrange("p a b -> p (a b)"),
                                        in_=x_in[tci, :, :])
                    xt_cache[tci] = xt
                return xt_cache[tci]

            def proj_qk_mm(tci, which, st):
                """q/k projection matmuls for one 512-token chunk."""
                xt = get_xt(tci)
                if tci + 1 < NTC:
                    get_xt(tci + 1)
                w_sb, b_sb = ((wq_sb, bq_sb) if which == "q"
                              else (wk_sb, bk_sb))
                pa = accps.tile([128, TC], fp32, tag="acc")
                for kt in range(KT):
                    nc.tensor.matmul(pa, lhsT=w_sb[:, kt, :],
                                     rhs=xt[:, kt, :],
                                     start=(kt == 0), stop=(kt == KT - 1))
                a_sb = work.tile([128, TC], bf16, tag="a_sb")
                nc.vector.tensor_scalar_add(a_sb, pa, b_sb)
                st[which] = a_sb

            def proj_qk_rope(tci, which, st):
                """RoPE for the chunk: rotation matmul + combine."""
                dest = get_qk(tci // CPB, which)
                a_sb = st.pop(which)
                pb = accps.tile([128, TC], fp32, tag="acc")
                nc.tensor.matmul(pb, lhsT=rt_sb, rhs=a_sb,
                                 start=True, stop=True)
                ssl = ds((tci * TC) % S, TC)
                t1 = work.tile([128, TC], bf16, tag="t1")
                nc.vector.tensor_mul(t1, a_sb, cos_sb[:, ssl])
                t2 = work.tile([128, TC], bf16, tag="t2")
                nc.vector.tensor_mul(t2, pb, sin_sb[:, ssl])
                nc.gpsimd.tensor_add(dest[:, ts(tci % CPB, TC)], t1, t2)

            def proj_v(tci):
                """v projection, transposed production (wv stationary),
                then DMA-transposed into natural layout vN on sync."""
                xt = get_xt(tci)
                pv = accps.tile([128, TC], fp32, tag="acc")
                for kt in range(KT):
                    nc.tensor.matmul(pv, lhsT=wv_sb[:, kt, :],
                                     rhs=xt[:, kt, :],
                                     start=(kt == 0), stop=(kt == KT - 1))
                vTst = vst.tile([128, TC], bf16, tag="vTst")
                nc.vector.tensor_scalar_add(vTst, pv, bv_sb)
                for sub in range(TC // 128):
                    tl = (tci % CPB) * (TC // 128) + sub
                    nc.sync.dma_start_transpose(
                        vN[tci // CPB][:, ds(192 * tl + 64, 128)],
                        vTst[:, ts(sub, 128)])

            def proj_pieces(b):
                out = []
                for cc in range(CPB):
                    tci = b * CPB + cc
                    st = {}
                    out.append(lambda t=tci, s=st: proj_qk_mm(t, "q", s))
                    out.append(lambda t=tci, s=st: proj_qk_mm(t, "k", s))
                    out.append(lambda t=tci, s=st: proj_qk_rope(t, "q", s))
                    out.append(lambda t=tci, s=st: proj_qk_rope(t, "k", s))
                    out.append(lambda t=tci: proj_v(t))
                return out

            # ---- phase 2a: scores + exp + mask for one (batch, head) ----
            def score_pieces(i, pt):
                """Closures, each = one <=1024-col psum group: matmul the
                row pieces intersecting the window, exp into pt, mask
                any diagonal blocks fully inside the window."""
                b, h = i // HPC, i % HPC
                hsl = ds(h * HD, HD)
                kTb = get_qk(b, "k")
                qTb = get_qk(b, "q")
                pieces = []
                x0 = 0
                while x0 < PTRI_W:
                    w = min(SGW, PTRI_W - x0)

                    def piece(x0=x0, w=w):
                        sc = bigps.tile([128, SGW], fp32, tag="big")
                        # rows intersecting flat-span window [x0, x0+w)
                        for kt in range(NQT):
                            r0, r1 = OFFS[kt], OFFS[kt] + (S - kt * 128)
                            lo, hi = max(r0, x0), min(r1, x0 + w)
                            if lo >= hi:
                                continue
                            q0 = kt * 128 + (lo - r0)
                            # split at 512-aligned psum columns: a matmul
                            # output must not cross a PSUM bank boundary
                            cuts = [lo]
                            nb = (lo - x0) // 512 * 512 + 512 + x0
                            while nb < hi:
                                cuts.append(nb)
                                nb += 512
                            cuts.append(hi)
                            for aa, bb in zip(cuts, cuts[1:]):
                                nc.tensor.matmul(
                                    sc[:, ds(aa - x0, bb - aa)],
                                    lhsT=kTb[hsl, ds(kt * 128, 128)],
                                    rhs=qTb[hsl, ds(q0 + (aa - lo), bb - aa)],
                                    start=True, stop=True)
                        nc.scalar.activation(
                            pt[:, ds(x0, w)], sc[:, 0:w],
                            mybir.ActivationFunctionType.Exp, scale=SCALE)
                        for kt in range(NQT):
                            if x0 <= OFFS[kt] and OFFS[kt] + 128 <= x0 + w:
                                dsl = ds(OFFS[kt], 128)
                                nc.gpsimd.tensor_mul(
                                    pt[:, dsl], pt[:, dsl], mask_sb)

                    pieces.append(piece)
                    x0 += w
                return pieces

            # ---- phase 2b: P@V (V stationary) + normalize into hT ----
            # Each P@V chunk's psum acc [128,512] (den rows + h rows) is
            # cast once to fp16 staging; den rows gather into [16,128],
            # DMA-transpose to [128,16], DVE reciprocal -> fp16, one DMA
            # scatters to rec_row [1,S] (q-order), and a K=1 ones-matmul
            # broadcasts 1/den to [64,512] psum for the normalize mult.
            def pv_pieces(i, pt):
                b, h = i // HPC, i % HPC
                st = {}
                pieces = []

                def mm_piece(c):
                    q0 = c * QC
                    acc = pvps.tile([128, QC], fp32, tag="pv")
                    kts = [kt for kt in range(NQT) if kt * 128 < q0 + QC]
                    for j, kt in enumerate(kts):
                        lo = max(kt * 128, q0)
                        w = q0 + QC - lo
                        lt = vN[b][:, ds(192 * kt + 128 * h, 128)]
                        nc.tensor.matmul(
                            acc[:, ds(lo - q0, w)], lhsT=lt,
                            rhs=pt[:, ds(OFFS[kt] + lo - kt * 128, w)],
                            start=(j == 0), stop=(j == len(kts) - 1))
                    if c == 0:
                        d16 = stg.tile([16, 128], fp16, tag="den16")
                        st["den16"] = d16
                    sacc = stg.tile([128, QC], fp16, tag="sacc", bufs=5)
                    nc.vector.tensor_copy(sacc, acc)
                    st[("hu", c)] = sacc[ds(64 - 64 * h, HD), :]
                    nc.sync.dma_start(
                        out=st["den16"][ds(4 * c, 4), :],
                        in_=sacc[ds(64 * h, 1), :])

                def rec_piece():
                    # den16 rows are already q-ordered: den16[R, m] =
                    # den(q = R*128 + m). Reciprocal in this layout (16
                    # lanes), then one partition-major DMA linearizes to
                    # rec_row[0, q]. bf16 rec_row: fp32 matmuls run in
                    # LOW_HIGH double-pass mode (~1us for N=512).
                    den32 = stg.tile([16, 128], fp32, tag="den32")
                    nc.vector.tensor_copy(den32, st["den16"])
                    rec32 = stg.tile([16, 128], fp32, tag="rec32")
                    nc.vector.reciprocal_approx_fast(rec32, den32)
                    rec16 = stg.tile([16, 128], fp16, tag="rec16")
                    nc.vector.tensor_copy(rec16, rec32)
                    rr = stg.tile([1, S], fp16, tag="rr", bufs=2)
                    nc.sync.dma_start(
                        out=bass.AP(tensor=rr.tensor, offset=rr.offset,
                                    ap=[[rr.ap[0][0], 1], [1, S]]),
                        in_=rec16[:, :])
                    for c in range(NPV):
                        recb = pvps.tile([HD, QC], fp32, tag="pv")
                        nc.tensor.matmul(
                            recb, lhsT=ones64f_sb[0:1, :],
                            rhs=rr[0:1, ds(c * QC, QC)],
                            start=True, stop=True)
                        nc.vector.tensor_mul(
                            hT[b][ds(h * HD, HD), ds(c * QC, QC)],
                            st[("hu", c)], recb)
                    if KDEBUG and i == 0:
                        nc.sync.dma_start(out=dbg_rr[:, :], in_=rr)

                for c in range(NPV):
                    pieces.append(lambda c=c: mm_piece(c))
                pieces.append(rec_piece)
                return pieces

            # ---- phase 3: output projection (partial, transposed) ----
            def oproj_piece(b, ft, cc, tail=False):
                po = accps.tile([128, 512], fp32, tag="acc")
                nc.tensor.matmul(
                    po, lhsT=wo_sb[:, ts(ft, 128)],
                    rhs=hT[b][:, ds(cc * 512, 512)],
                    start=True, stop=True)
                ostage = work.tile([128, 512], bf16, tag="ostage", bufs=4)
                nc.vector.tensor_copy(ostage, po)
                nc.gpsimd.dma_start(
                    out=out_d[ft, b * (S // 512) + cc, :, :], in_=ostage)

            def oproj_pieces(b, fts, tail=False):
                def two(bb, f, c0):
                    oproj_piece(bb, f, c0, tail=tail)
                    oproj_piece(bb, f, c0 + 1, tail=tail)
                return [lambda f=ft, c=cc, bb=b: two(bb, f, c)
                        for ft in fts for cc in (0, 2)]

            # ---- emission schedule ----
            def interleave(main, others, ratio=None):
                """Emit main pieces with `others` spliced evenly."""
                if ratio is None:
                    ratio = max(1, len(main) // max(1, len(others)))
                oi = 0
                for n, m in enumerate(main):
                    m()
                    if n % ratio == ratio - 1 and oi < len(others):
                        others[oi]()
                        oi += 1
                for o in others[oi:]:
                    o()

            # startup: batch-0 projection
            for p in proj_pieces(0):
                p()

            nprob = B * HPC
            pts = {}
            pvq = []      # pending P@V pieces (from previous problem)
            pending = []  # filler queue, carried across iterations
            for i in range(nprob + 1):
                # projection for batch i//2+1 split over iterations 2b-2,
                # 2b-1; O-proj for batch (i-3)//2 split over 2b+3, 2b+4.
                # Proj pieces go first: their casts feed the PE (rope
                # matmuls) soonest.
                bb = i // 2 + 1
                half = i % 2
                if bb < B:
                    pending += proj_pieces(bb)[10 * half:10 * half + 10]
                pending += pvq
                pvq = []
                if i >= 3:
                    ob = (i - 3) // 2
                    ohalf = (i - 3) % 2
                    pending += oproj_pieces(ob, range(4 * ohalf, 4 * ohalf + 4))
                if i == nprob:   # epilogue: last batch's O-proj
                    pending += oproj_pieces(B - 1, range(8), tail=True)
                if i < nprob:
                    pt = ptri_pool.tile([128, PTRI_W], bf16, tag="pt")
                    pts[i] = pt
                    wins = score_pieces(i, pt)
                    nwin = len(wins)
                    for wi, wpc in enumerate(wins):
                        wpc()
                        # drain fillers proportionally; leftovers carry
                        # into the next iteration instead of dumping here
                        want = (len(pending) + nwin - wi - 1) // (nwin - wi)
                        for _ in range(min(want, 2)):
                            if pending:
                                pending.pop(0)()
                    pvq = pv_pieces(i, pt)
                else:
                    for p in pending:
                        p()
                    pending = []
            # pv of the last problem ran inside the epilogue fillers
            if KDEBUG:
                for bb in range(B):
                    nc.sync.dma_start(out=dbg_q[:, ts(bb, S)],
                                      in_=get_qk(bb, "q"))
                    nc.sync.dma_start(out=dbg_k[:, ts(bb, S)],
                                      in_=get_qk(bb, "k"))
                    nc.sync.dma_start(out=dbg_h[:, ts(bb, S)], in_=hT[bb])

    nc.compile()
    return nc


_NC_CACHE = None


def _get_nc():
    global _NC_CACHE
    if _NC_CACHE is None:
        _NC_CACHE = _build_nc()
    return _NC_CACHE


def build_in_maps(x, positions, Wqkv, bqkv, Wo, bo):
    xT = x.reshape(T, D).T.astype(BF16)            # [D, T]
    # chunk-block layout [tci, p, kt*512]: contiguous 4KB runs per partition
    xblk = np.ascontiguousarray(
        xT.reshape(KT, 128, NTC, TC).transpose(2, 1, 0, 3).reshape(NTC, 128, KT * TC))
    # RoPE tables, layout matched to the stacked-heads partition dim:
    # partition p -> head-local pair i = (p % 64) // 2
    i_of_p = (np.arange(PC) % HD) // 2
    invf = 1.0 / (ROPE_THETA ** (2.0 * i_of_p / HD))          # [128]
    ang = invf[:, None] * np.asarray(positions[0])[None, :]   # [128, S]
    cost = np.ascontiguousarray(np.cos(ang)).astype(BF16)
    sint = np.ascontiguousarray(np.sin(ang)).astype(BF16)
    in_maps = []
    for c in range(NCORES):
        r0 = c * PC
        wq = np.ascontiguousarray(Wqkv[r0:r0 + PC, :].T).astype(BF16)
        wk = np.ascontiguousarray(Wqkv[D + r0:D + r0 + PC, :].T).astype(BF16)
        wv = np.ascontiguousarray(Wqkv[2 * D + r0:2 * D + r0 + PC, :].T).astype(BF16)
        wo = np.ascontiguousarray(Wo[:, r0:r0 + PC].T).astype(BF16)
        in_maps.append({
            "x": xblk, "cost": cost, "sint": sint,
            "wq": wq, "wk": wk, "wv": wv, "wo": wo,
            "bq": bqkv[r0:r0 + PC].astype(np.float32),
            "bk": bqkv[D + r0:D + r0 + PC].astype(np.float32),
            "bv": bqkv[2 * D + r0:2 * D + r0 + PC].astype(np.float32),
        })
    return in_maps


def assemble_out(res, bo):
    acc = res.results[0]["out"].astype(np.float32)
    for c in range(1, NCORES):
        acc += res.results[c]["out"].astype(np.float32)
    # [KT, T//512, 128, 512] -> [D, T]
    full = acc.transpose(0, 2, 1, 3).reshape(D, T)
    out = full + bo[:, None].astype(np.float32)
    return np.ascontiguousarray(out.T).reshape(B, S, D)


def kernel(x, positions, Wqkv, bqkv, Wo, bo):
    x = np.asarray(x)
    positions = np.asarray(positions)
    Wqkv = np.asarray(Wqkv)
    bqkv = np.asarray(bqkv)
    Wo = np.asarray(Wo)
    bo = np.asarray(bo)
    nc = _get_nc()
    in_maps = build_in_maps(x, positions, Wqkv, bqkv, Wo, bo)
    res = run_bass_kernel_spmd(nc, in_maps, core_ids=list(range(NCORES)))
    return assemble_out(res, bo)


# revision 30
# speedup vs baseline: 1.3610x; 1.0289x over previous
"""Multi-head self-attention (RoPE, causal) Trainium2 Bass kernel.

Problem: B=4, S=2048, D=1024, H=16 heads, hd=64, fused QKV + RoPE +
causal softmax attention + output projection (torch-Linear convention).

Sharding: Megatron-style tensor parallel over heads. Each of the 8
NeuronCores owns 2 heads: it projects the full token stream through its
128-row slices of Wq/Wk/Wv, applies RoPE, runs causal attention for its
2 heads x 4 batches, and computes a partial output projection
h_core @ Wo[:, core_slice].T  (transposed layout). The host sums the 8
partial outputs and adds the output bias.

v4 design notes (vs v2 baseline at ~601us; this version ~490us):
  - RoPE cos/sin tables are computed host-side in build_in_maps from the
    positions input (fp64 numpy -> bf16) and DMA'd in. This removed the
    on-device range-reduced Sin pipeline (22us of DVE at startup + 16
    activations) and cut the end-to-end relative error 1.8% -> 0.76%
    (the bf16 Sin spline was the dominant error source).
  - Score psum groups are 1024 wide (2 banks, double buffered = 4
    banks); each exp ACTIVATE covers 1024 columns, halving scalar-engine
    per-instruction overhead (136 calls).
  - Softmax denominator path: den rows gather to a q-ordered [16,128]
    fp16 tile, fp32 reciprocal_approx_fast in that layout, one
    partition-major DMA linearizes to rec_row[1,S] fp16, and a K=1
    ones-matmul broadcasts 1/den to [64,512] psum (rides the pv psum
    pool). This replaced the ejs/idf selector matmuls (128 MMs), a PE
    transpose, and two dedicated psum banks.
  - P@V staging: one [128,512] fp16 CAST per chunk captures h values
    AND den rows; hu/den are views into it.
  - Emission scheduling: proj chunk pieces are decoupled
    ([mm q][mm k][rope q][rope k][v]) so each psum group's DVE cast has
    ~2 pieces of slack before its consumer matmul; fillers (proj of the
    next batch, P@V of the previous problem, o-proj) live in a queue
    that drains proportionally between score windows and carries across
    iterations, keeping the PE fed so the HAM clock gate stays warm.
  - RoPE final add and the causal masks run on gpsimd; o-proj and P@V
    psum->sbuf casts run on DVE; the scalar engine runs only Exp.
  - qT/kT residents rotate through 2 buffers; x chunk 0 is DMA'd in 8
    per-ktile slices so the first projection matmul starts ~5us earlier.
"""

import os
import sys

for _p in ("/opt/trn_rl_repo",):
    if os.path.isdir(_p) and _p not in sys.path:
        sys.path.append(_p)

import math

import ml_dtypes
import numpy as np

import concourse.bass as bass
import concourse.mybir as mybir
import concourse.tile as tile
from concourse import bacc
from concourse.bass import ts, ds
from concourse.bass_utils import run_bass_kernel_spmd

BF16 = ml_dtypes.bfloat16

B = 4
S = 2048
D = 1024
H = 16
HD = 64
NCORES = 8
HPC = H // NCORES          # heads per core = 2
PC = HPC * HD              # partition rows per core's heads = 128
T = B * S                  # 8192 tokens
KT = D // 128              # f_in k-tiles = 8
NTOK = T // 128            # 64 token tiles of 128
SCALE = 1.0 / math.sqrt(HD)
ROPE_THETA = 10000.0

TWO_PI = 2.0 * math.pi
INV_2PI = 1.0 / TWO_PI
MAGIC = 12582912.0         # 1.5 * 2**23, float32 round-to-nearest trick
HALF_PI = math.pi / 2.0

NQT = S // 128             # 16 q/k tiles per sequence
# triangular packing offsets for P_T: row kt covers q in [kt*128, S)
OFFS = [0] * NQT
for _kt in range(1, NQT):
    OFFS[_kt] = OFFS[_kt - 1] + (S - (_kt - 1) * 128)
PTRI_W = OFFS[-1] + (S - (NQT - 1) * 128)   # 17408

TC = 512                   # token chunk for projections
NTC = T // TC              # 16
CPB = S // TC              # proj chunks per batch = 4
QC = 512                   # P@V q-chunk width
NPV = S // QC              # P@V chunks per problem = 4
SGW = 1024                 # scores psum group width (2 banks)


def _build_nc():
    nc = bacc.Bacc("TRN2", target_bir_lowering=False, debug=False,
                   num_devices=NCORES)
    dt = mybir.dt

    # ---- I/O ----
    x_in = nc.dram_tensor("x", [NTC, 128, KT * TC], dt.bfloat16,
                          kind="ExternalInput")
    cos_in = nc.dram_tensor("cost", [128, S], dt.bfloat16, kind="ExternalInput")
    sin_in = nc.dram_tensor("sint", [128, S], dt.bfloat16, kind="ExternalInput")
    wq_in = nc.dram_tensor("wq", [D, PC], dt.bfloat16, kind="ExternalInput")
    wk_in = nc.dram_tensor("wk", [D, PC], dt.bfloat16, kind="ExternalInput")
    wv_in = nc.dram_tensor("wv", [D, PC], dt.bfloat16, kind="ExternalInput")
    wo_in = nc.dram_tensor("wo", [PC, D], dt.bfloat16, kind="ExternalInput")
    bq_in = nc.dram_tensor("bq", [PC], dt.float32, kind="ExternalInput")
    bk_in = nc.dram_tensor("bk", [PC], dt.float32, kind="ExternalInput")
    bv_in = nc.dram_tensor("bv", [PC], dt.float32, kind="ExternalInput")
    out_d = nc.dram_tensor("out", [KT, T // 512, 128, 512], dt.bfloat16,
                           kind="ExternalOutput")
    KDEBUG = os.environ.get("KDEBUG") == "1"
    if KDEBUG:
        dbg_q = nc.dram_tensor("dbg_q", [128, T], dt.bfloat16, kind="ExternalOutput")
        dbg_k = nc.dram_tensor("dbg_k", [128, T], dt.bfloat16, kind="ExternalOutput")
        dbg_h = nc.dram_tensor("dbg_h", [128, T], dt.bfloat16, kind="ExternalOutput")
        dbg_pt = nc.dram_tensor("dbg_pt", [128, PTRI_W], dt.bfloat16,
                                kind="ExternalOutput")
        dbg_rr = nc.dram_tensor("dbg_rr", [1, S], dt.float16,
                                kind="ExternalOutput")

    # ---- inline constants ----
    # RT = R.T where (R @ q)[2i] = -q[2i+1], (R @ q)[2i+1] = q[2i],
    # block-diagonal over the 2 stacked heads (structure identical).
    r = np.zeros((PC, PC), dtype=np.float32)
    for h in range(HPC):
        for i in range(HD // 2):
            r[h * HD + 2 * i, h * HD + 2 * i + 1] = -1.0
            r[h * HD + 2 * i + 1, h * HD + 2 * i] = 1.0
    rt_np = np.ascontiguousarray(r.T).astype(BF16)
    # causal mask for diagonal scoresT blocks: keep k_local <= q_local
    mask_np = np.tril(np.ones((128, 128), dtype=np.float32)).T.astype(BF16)
    rt_d = nc.inline_tensor(rt_np, "rt_c")
    ones64_np = np.ones((1, HD), dtype=np.float16)
    ones64_d = nc.inline_tensor(ones64_np, "ones64_c")
    mask_d = nc.inline_tensor(mask_np, "mask_c")

    fp32 = dt.float32
    bf16 = dt.bfloat16
    fp16 = dt.float16

    with tile.TileContext(nc) as tc:
        with (
            tc.tile_pool(name="consts", bufs=1) as consts,
            tc.tile_pool(name="resid", bufs=1) as resid,
            tc.tile_pool(name="xp", bufs=2) as xp,
            tc.tile_pool(name="work", bufs=2) as work,
            tc.tile_pool(name="vst", bufs=2) as vst,
            tc.tile_pool(name="stg", bufs=2) as stg,
            tc.tile_pool(name="ptri", bufs=2) as ptri_pool,
            tc.tile_pool(name="bigps", bufs=2, space="PSUM") as bigps,
            tc.tile_pool(name="pvps", bufs=2, space="PSUM") as pvps,
            tc.tile_pool(name="accps", bufs=2, space="PSUM") as accps,
        ):
            # ---- load constants / weights to SBUF ----
            # x chunk 0 DMA first so the PE can start ASAP; weights on
            # separate queues so wq doesn't queue behind everything.
            xt0 = xp.tile([128, KT, TC], bf16, tag="xt")
            for _kt in range(KT):
                nc.scalar.dma_start(out=xt0[:, _kt, :],
                                    in_=x_in[0, :, ds(_kt * TC, TC)])

            wq_sb = consts.tile([128, KT, PC], bf16, tag="wq")
            wk_sb = consts.tile([128, KT, PC], bf16, tag="wk")
            wv_sb = consts.tile([128, KT, PC], bf16, tag="wv")
            nc.sync.dma_start(
                out=wq_sb, in_=wq_in.ap().rearrange("(kt p) m -> p kt m", p=128))
            nc.gpsimd.dma_start(
                out=wk_sb, in_=wk_in.ap().rearrange("(kt p) m -> p kt m", p=128))
            nc.scalar.dma_start(
                out=wv_sb, in_=wv_in.ap().rearrange("(kt p) m -> p kt m", p=128))
            wo_sb = consts.tile([128, D], bf16, tag="wo")
            nc.gpsimd.dma_start(out=wo_sb, in_=wo_in[:, :])
            rt_sb = consts.tile([128, 128], bf16, tag="rt")
            nc.sync.dma_start(out=rt_sb, in_=rt_d[:, :])
            mask_sb = consts.tile([128, 128], bf16, tag="mask")
            nc.sync.dma_start(out=mask_sb, in_=mask_d[:, :])
            ones64f_sb = consts.tile([1, HD], fp16, tag="ones64")
            nc.sync.dma_start(out=ones64f_sb, in_=ones64_d[:, :])
            bq_sb = consts.tile([128, 1], fp32, tag="bq")
            nc.sync.dma_start(out=bq_sb, in_=bq_in.ap().rearrange("(p o) -> p o", o=1))
            bk_sb = consts.tile([128, 1], fp32, tag="bk")
            nc.sync.dma_start(out=bk_sb, in_=bk_in.ap().rearrange("(p o) -> p o", o=1))
            bv_sb = consts.tile([128, 1], fp32, tag="bv")
            nc.sync.dma_start(out=bv_sb, in_=bv_in.ap().rearrange("(p o) -> p o", o=1))

            # ---- residents ----
            # qT/kT rotate through 2 buffers (proj of batch b+1 overlaps
            # scores of batch b); hT stays per-batch (read by o-proj two
            # problems later).
            qkh = {}

            def get_qk(bb, which):
                if (bb, which) not in qkh:
                    t = resid.tile([128, S], bf16, tag=which, bufs=2,
                                   name=f"{which}{bb}")
                    qkh[(bb, which)] = t
                return qkh[(bb, which)]

            hT = []
            for bb in range(B):
                th = resid.tile([128, S], bf16, tag=f"hT{bb}")
                hT.append(th)
            # v natural as repeating [ones(64) | d_h0(64) | d_h1(64)]
            # 192-col blocks (plus one trailing ones block): head0's P@V
            # lhsT is [ones|d0] (den in psum rows 0-63, h in 64-127) and
            # head1's is [d1|ones-of-next-block] (h in 0-63, den 64-127) --
            # both plain contiguous 128-col slices.
            NTB = NTOK // B            # 16 tok tiles per batch
            VW = NTB * 192 + 64
            vN = []
            for bb in range(B):
                tv = resid.tile([128, VW], bf16, tag=f"vN{bb}")
                vN.append(tv)
                nc.vector.memset(
                    bass.AP(tensor=tv.tensor, offset=tv.offset,
                            ap=[tv.ap[0], [192, NTB + 1], [1, 64]]), 1.0)
            # RoPE cos/sin tables [128, S] bf16, computed host-side
            cos_sb = consts.tile([128, S], bf16, tag="cosT")
            nc.gpsimd.dma_start(out=cos_sb, in_=cos_in[:, :])
            sin_sb = consts.tile([128, S], bf16, tag="sinT")
            nc.gpsimd.dma_start(out=sin_sb, in_=sin_in[:, :])

            # ---- phase 1: QKV projection + RoPE, per token chunk ----
            xt_cache = {0: xt0}

            def get_xt(tci):
                if tci not in xt_cache:
                    xt = xp.tile([128, KT, TC], bf16, tag="xt")
                    nc.scalar.dma_start(out=xt.rearrange("p a b -> p (a b)"),
                                        in_=x_in[tci, :, :])
                    xt_cache[tci] = xt
                return xt_cache[tci]

            def proj_qk_mm(tci, which, st):
                """q/k projection matmuls for one 512-token chunk."""
                xt = get_xt(tci)
                if tci + 1 < NTC:
                    get_xt(tci + 1)
                w_sb, b_sb = ((wq_sb, bq_sb) if which == "q"
                              else (wk_sb, bk_sb))
                pa = accps.tile([128, TC], fp32, tag="acc")
                for kt in range(KT):
                    nc.tensor.matmul(pa, lhsT=w_sb[:, kt, :],
                                     rhs=xt[:, kt, :],
                                     start=(kt == 0), stop=(kt == KT - 1))
                a_sb = work.tile([128, TC], bf16, tag="a_sb", bufs=4)
                nc.vector.tensor_scalar_add(a_sb, pa, b_sb)
                st[which] = a_sb

            def proj_qk_rope(tci, which, st):
                """RoPE for the chunk: rotation matmul + combine."""
                dest = get_qk(tci // CPB, which)
                a_sb = st.pop(which)
                pb = accps.tile([128, TC], fp32, tag="acc")
                nc.tensor.matmul(pb, lhsT=rt_sb, rhs=a_sb,
                                 start=True, stop=True)
                ssl = ds((tci * TC) % S, TC)
                t1 = work.tile([128, TC], bf16, tag="t1", bufs=4)
                nc.vector.tensor_mul(t1, a_sb, cos_sb[:, ssl])
                t2 = work.tile([128, TC], bf16, tag="t2", bufs=4)
                nc.vector.tensor_mul(t2, pb, sin_sb[:, ssl])
                nc.gpsimd.tensor_add(dest[:, ts(tci % CPB, TC)], t1, t2)

            def proj_v(tci):
                """v projection, transposed production (wv stationary),
                then DMA-transposed into natural layout vN on sync."""
                xt = get_xt(tci)
                pv = accps.tile([128, TC], fp32, tag="acc")
                for kt in range(KT):
                    nc.tensor.matmul(pv, lhsT=wv_sb[:, kt, :],
                                     rhs=xt[:, kt, :],
                                     start=(kt == 0), stop=(kt == KT - 1))
                vTst = vst.tile([128, TC], bf16, tag="vTst", bufs=4)
                nc.vector.tensor_scalar_add(vTst, pv, bv_sb)
                for sub in range(TC // 128):
                    tl = (tci % CPB) * (TC // 128) + sub
                    nc.sync.dma_start_transpose(
                        vN[tci // CPB][:, ds(192 * tl + 64, 128)],
                        vTst[:, ts(sub, 128)])

            def proj_pieces(b):
                out = []
                for cc in range(CPB):
                    tci = b * CPB + cc
                    st = {}
                    out.append(lambda t=tci, s=st: proj_qk_mm(t, "q", s))
                    out.append(lambda t=tci, s=st: proj_qk_mm(t, "k", s))
                    out.append(lambda t=tci, s=st: proj_qk_rope(t, "q", s))
                    out.append(lambda t=tci, s=st: proj_qk_rope(t, "k", s))
                    out.append(lambda t=tci: proj_v(t))
                return out

            # ---- phase 2a: scores + exp + mask for one (batch, head) ----
            def score_pieces(i, pt):
                """Closures, each = one <=1024-col psum group: matmul the
                row pieces intersecting the window, exp into pt, mask
                any diagonal blocks fully inside the window."""
                b, h = i // HPC, i % HPC
                hsl = ds(h * HD, HD)
                kTb = get_qk(b, "k")
                qTb = get_qk(b, "q")
                pieces = []
                x0 = 0
                while x0 < PTRI_W:
                    w = min(SGW, PTRI_W - x0)

                    def piece(x0=x0, w=w):
                        sc = bigps.tile([128, SGW], fp32, tag="big")
                        # rows intersecting flat-span window [x0, x0+w)
                        for kt in range(NQT):
                            r0, r1 = OFFS[kt], OFFS[kt] + (S - kt * 128)
                            lo, hi = max(r0, x0), min(r1, x0 + w)
                            if lo >= hi:
                                continue
                            q0 = kt * 128 + (lo - r0)
                            # split at 512-aligned psum columns: a matmul
                            # output must not cross a PSUM bank boundary
                            cuts = [lo]
                            nb = (lo - x0) // 512 * 512 + 512 + x0
                            while nb < hi:
                                cuts.append(nb)
                                nb += 512
                            cuts.append(hi)
                            for aa, bb in zip(cuts, cuts[1:]):
                                nc.tensor.matmul(
                                    sc[:, ds(aa - x0, bb - aa)],
                                    lhsT=kTb[hsl, ds(kt * 128, 128)],
                                    rhs=qTb[hsl, ds(q0 + (aa - lo), bb - aa)],
                                    start=True, stop=True)
                        nc.scalar.activation(
                            pt[:, ds(x0, w)], sc[:, 0:w],
                            mybir.ActivationFunctionType.Exp, scale=SCALE)
                        for kt in range(NQT):
                            if x0 <= OFFS[kt] and OFFS[kt] + 128 <= x0 + w:
                                dsl = ds(OFFS[kt], 128)
                                nc.gpsimd.tensor_mul(
                                    pt[:, dsl], pt[:, dsl], mask_sb)

                    pieces.append(piece)
                    x0 += w
                return pieces

            # ---- phase 2b: P@V (V stationary) + normalize into hT ----
            # Each P@V chunk's psum acc [128,512] (den rows + h rows) is
            # cast once to fp16 staging; den rows gather into [16,128],
            # DMA-transpose to [128,16], DVE reciprocal -> fp16, one DMA
            # scatters to rec_row [1,S] (q-order), and a K=1 ones-matmul
            # broadcasts 1/den to [64,512] psum for the normalize mult.
            def pv_pieces(i, pt):
                b, h = i // HPC, i % HPC
                st = {}
                pieces = []

                def mm_piece(c):
                    q0 = c * QC
                    acc = pvps.tile([128, QC], fp32, tag="pv")
                    kts = [kt for kt in range(NQT) if kt * 128 < q0 + QC]
                    for j, kt in enumerate(kts):
                        lo = max(kt * 128, q0)
                        w = q0 + QC - lo
                        lt = vN[b][:, ds(192 * kt + 128 * h, 128)]
                        nc.tensor.matmul(
                            acc[:, ds(lo - q0, w)], lhsT=lt,
                            rhs=pt[:, ds(OFFS[kt] + lo - kt * 128, w)],
                            start=(j == 0), stop=(j == len(kts) - 1))
                    if c == 0:
                        d16 = stg.tile([16, 128], fp16, tag="den16")
                        st["den16"] = d16
                    sacc = stg.tile([128, QC], fp16, tag="sacc", bufs=5)
                    nc.vector.tensor_copy(sacc, acc)
                    st[("hu", c)] = sacc[ds(64 - 64 * h, HD), :]
                    nc.sync.dma_start(
                        out=st["den16"][ds(4 * c, 4), :],
                        in_=sacc[ds(64 * h, 1), :])

                def rec_piece():
                    # den16 rows are already q-ordered: den16[R, m] =
                    # den(q = R*128 + m). Reciprocal in this layout (16
                    # lanes), then one partition-major DMA linearizes to
                    # rec_row[0, q]. bf16 rec_row: fp32 matmuls run in
                    # LOW_HIGH double-pass mode (~1us for N=512).
                    den32 = stg.tile([16, 128], fp32, tag="den32")
                    nc.vector.tensor_copy(den32, st["den16"])
                    rec32 = stg.tile([16, 128], fp32, tag="rec32")
                    nc.vector.reciprocal_approx_fast(rec32, den32)
                    rec16 = stg.tile([16, 128], fp16, tag="rec16")
                    nc.vector.tensor_copy(rec16, rec32)
                    rr = stg.tile([1, S], fp16, tag="rr", bufs=2)
                    nc.sync.dma_start(
                        out=bass.AP(tensor=rr.tensor, offset=rr.offset,
                                    ap=[[rr.ap[0][0], 1], [1, S]]),
                        in_=rec16[:, :])
                    for c in range(NPV):
                        recb = pvps.tile([HD, QC], fp32, tag="pv")
                        nc.tensor.matmul(
                            recb, lhsT=ones64f_sb[0:1, :],
                            rhs=rr[0:1, ds(c * QC, QC)],
                            start=True, stop=True)
                        nc.vector.tensor_mul(
                            hT[b][ds(h * HD, HD), ds(c * QC, QC)],
                            st[("hu", c)], recb)
                    if KDEBUG and i == 0:
                        nc.sync.dma_start(out=dbg_rr[:, :], in_=rr)

                for c in range(NPV):
                    pieces.append(lambda c=c: mm_piece(c))
                pieces.append(rec_piece)
                return pieces

            # ---- phase 3: output projection (partial, transposed) ----
            def oproj_piece(b, ft, cc, tail=False):
                po = accps.tile([128, 512], fp32, tag="acc")
                nc.tensor.matmul(
                    po, lhsT=wo_sb[:, ts(ft, 128)],
                    rhs=hT[b][:, ds(cc * 512, 512)],
                    start=True, stop=True)
                ostage = work.tile([128, 512], bf16, tag="ostage", bufs=4)
                nc.vector.tensor_copy(ostage, po)
                nc.gpsimd.dma_start(
                    out=out_d[ft, b * (S // 512) + cc, :, :], in_=ostage)

            def oproj_pieces(b, fts, tail=False):
                def two(bb, f, c0):
                    oproj_piece(bb, f, c0, tail=tail)
                    oproj_piece(bb, f, c0 + 1, tail=tail)
                return [lambda f=ft, c=cc, bb=b: two(bb, f, c)
                        for ft in fts for cc in (0, 2)]

            # ---- emission schedule ----
            def interleave(main, others, ratio=None):
                """Emit main pieces with `others` spliced evenly."""
                if ratio is None:
                    ratio = max(1, len(main) // max(1, len(others)))
                oi = 0
                for n, m in enumerate(main):
                    m()
                    if n % ratio == ratio - 1 and oi < len(others):
                        others[oi]()
                        oi += 1
                for o in others[oi:]:
                    o()

            # startup: batch-0 projection
            for p in proj_pieces(0):
                p()

            nprob = B * HPC
            pts = {}
            pvq = []      # pending P@V pieces (from previous problem)
            pending = []  # filler queue, carried across iterations
            for i in range(nprob + 1):
                # projection for batch i//2+1 split over iterations 2b-2,
                # 2b-1; O-proj for batch (i-3)//2 split over 2b+3, 2b+4.
                # Proj pieces go first: their casts feed the PE (rope
                # matmuls) soonest.
                bb = i // 2 + 1
                half = i % 2
                if bb < B:
                    pending += proj_pieces(bb)[10 * half:10 * half + 10]
                pending += pvq
                pvq = []
                if i >= 3:
                    ob = (i - 3) // 2
                    ohalf = (i - 3) % 2
                    pending += oproj_pieces(ob, range(4 * ohalf, 4 * ohalf + 4))
                if i == nprob:   # epilogue: last batch's O-proj
                    pending += oproj_pieces(B - 1, range(8), tail=True)
                if i < nprob:
                    pt = ptri_pool.tile([128, PTRI_W], bf16, tag="pt")
                    pts[i] = pt
                    wins = score_pieces(i, pt)
                    nwin = len(wins)
                    for wi, wpc in enumerate(wins):
                        wpc()
                        # drain fillers proportionally; leftovers carry
                        # into the next iteration instead of dumping here
                        want = (len(pending) + nwin - wi - 1) // (nwin - wi)
                        for _ in range(min(want, 2)):
                            if pending:
                                pending.pop(0)()
                    pvq = pv_pieces(i, pt)
                else:
                    for p in pending:
                        p()
                    pending = []
            # pv of the last problem ran inside the epilogue fillers
            if KDEBUG:
                for bb in range(B):
                    nc.sync.dma_start(out=dbg_q[:, ts(bb, S)],
                                      in_=get_qk(bb, "q"))
                    nc.sync.dma_start(out=dbg_k[:, ts(bb, S)],
                                      in_=get_qk(bb, "k"))
                    nc.sync.dma_start(out=dbg_h[:, ts(bb, S)], in_=hT[bb])

    nc.compile()
    return nc


_NC_CACHE = None


def _get_nc():
    global _NC_CACHE
    if _NC_CACHE is None:
        _NC_CACHE = _build_nc()
    return _NC_CACHE


def build_in_maps(x, positions, Wqkv, bqkv, Wo, bo):
    xT = x.reshape(T, D).T.astype(BF16)            # [D, T]
    # chunk-block layout [tci, p, kt*512]: contiguous 4KB runs per partition
    xblk = np.ascontiguousarray(
        xT.reshape(KT, 128, NTC, TC).transpose(2, 1, 0, 3).reshape(NTC, 128, KT * TC))
    # RoPE tables, layout matched to the stacked-heads partition dim:
    # partition p -> head-local pair i = (p % 64) // 2
    i_of_p = (np.arange(PC) % HD) // 2
    invf = 1.0 / (ROPE_THETA ** (2.0 * i_of_p / HD))          # [128]
    ang = invf[:, None] * np.asarray(positions[0])[None, :]   # [128, S]
    cost = np.ascontiguousarray(np.cos(ang)).astype(BF16)
    sint = np.ascontiguousarray(np.sin(ang)).astype(BF16)
    in_maps = []
    for c in range(NCORES):
        r0 = c * PC
        wq = np.ascontiguousarray(Wqkv[r0:r0 + PC, :].T).astype(BF16)
        wk = np.ascontiguousarray(Wqkv[D + r0:D + r0 + PC, :].T).astype(BF16)
        wv = np.ascontiguousarray(Wqkv[2 * D + r0:2 * D + r0 + PC, :].T).astype(BF16)
        wo = np.ascontiguousarray(Wo[:, r0:r0 + PC].T).astype(BF16)
        in_maps.append({
            "x": xblk, "cost": cost, "sint": sint,
            "wq": wq, "wk": wk, "wv": wv, "wo": wo,
            "bq": bqkv[r0:r0 + PC].astype(np.float32),
            "bk": bqkv[D + r0:D + r0 + PC].astype(np.float32),
            "bv": bqkv[2 * D + r0:2 * D + r0 + PC].astype(np.float32),
        })
    return in_maps


def assemble_out(res, bo):
    acc = res.results[0]["out"].astype(np.float32)
    for c in range(1, NCORES):
        acc += res.results[c]["out"].astype(np.float32)
    # [KT, T//512, 128, 512] -> [D, T]
    full = acc.transpose(0, 2, 1, 3).reshape(D, T)
    out = full + bo[:, None].astype(np.float32)
    return np.ascontiguousarray(out.T).reshape(B, S, D)


def kernel(x, positions, Wqkv, bqkv, Wo, bo):
    x = np.asarray(x)
    positions = np.asarray(positions)
    Wqkv = np.asarray(Wqkv)
    bqkv = np.asarray(bqkv)
    Wo = np.asarray(Wo)
    bo = np.asarray(bo)
    nc = _get_nc()
    in_maps = build_in_maps(x, positions, Wqkv, bqkv, Wo, bo)
    res = run_bass_kernel_spmd(nc, in_maps, core_ids=list(range(NCORES)))
    return assemble_out(res, bo)


# revision 31
# speedup vs baseline: 1.3656x; 1.0034x over previous
"""Multi-head self-attention (RoPE, causal) Trainium2 Bass kernel.

Problem: B=4, S=2048, D=1024, H=16 heads, hd=64, fused QKV + RoPE +
causal softmax attention + output projection (torch-Linear convention).

Sharding: Megatron-style tensor parallel over heads. Each of the 8
NeuronCores owns 2 heads: it projects the full token stream through its
128-row slices of Wq/Wk/Wv, applies RoPE, runs causal attention for its
2 heads x 4 batches, and computes a partial output projection
h_core @ Wo[:, core_slice].T  (transposed layout). The host sums the 8
partial outputs and adds the output bias.

v4 design notes (vs v2 baseline at ~601us; this version ~490us):
  - RoPE cos/sin tables are computed host-side in build_in_maps from the
    positions input (fp64 numpy -> bf16) and DMA'd in. This removed the
    on-device range-reduced Sin pipeline (22us of DVE at startup + 16
    activations) and cut the end-to-end relative error 1.8% -> 0.76%
    (the bf16 Sin spline was the dominant error source).
  - Score psum groups are 1024 wide (2 banks, double buffered = 4
    banks); each exp ACTIVATE covers 1024 columns, halving scalar-engine
    per-instruction overhead (136 calls).
  - Softmax denominator path: den rows gather to a q-ordered [16,128]
    fp16 tile, fp32 reciprocal_approx_fast in that layout, one
    partition-major DMA linearizes to rec_row[1,S] fp16, and a K=1
    ones-matmul broadcasts 1/den to [64,512] psum (rides the pv psum
    pool). This replaced the ejs/idf selector matmuls (128 MMs), a PE
    transpose, and two dedicated psum banks.
  - P@V staging: one [128,512] fp16 CAST per chunk captures h values
    AND den rows; hu/den are views into it.
  - Emission scheduling: proj chunk pieces are decoupled
    ([mm q][mm k][rope q][rope k][v]) so each psum group's DVE cast has
    ~2 pieces of slack before its consumer matmul; fillers (proj of the
    next batch, P@V of the previous problem, o-proj) live in a queue
    that drains proportionally between score windows and carries across
    iterations, keeping the PE fed so the HAM clock gate stays warm.
  - RoPE final add and the causal masks run on gpsimd; o-proj and P@V
    psum->sbuf casts run on DVE; the scalar engine runs only Exp.
  - qT/kT residents rotate through 2 buffers; x chunk 0 is DMA'd in 8
    per-ktile slices so the first projection matmul starts ~5us earlier.
"""

import os
import sys

for _p in ("/opt/trn_rl_repo",):
    if os.path.isdir(_p) and _p not in sys.path:
        sys.path.append(_p)

import math

import ml_dtypes
import numpy as np

import concourse.bass as bass
import concourse.mybir as mybir
import concourse.tile as tile
from concourse import bacc
from concourse.bass import ts, ds
from concourse.bass_utils import run_bass_kernel_spmd

BF16 = ml_dtypes.bfloat16

B = 4
S = 2048
D = 1024
H = 16
HD = 64
NCORES = 8
HPC = H // NCORES          # heads per core = 2
PC = HPC * HD              # partition rows per core's heads = 128
T = B * S                  # 8192 tokens
KT = D // 128              # f_in k-tiles = 8
NTOK = T // 128            # 64 token tiles of 128
SCALE = 1.0 / math.sqrt(HD)
ROPE_THETA = 10000.0

TWO_PI = 2.0 * math.pi
INV_2PI = 1.0 / TWO_PI
MAGIC = 12582912.0         # 1.5 * 2**23, float32 round-to-nearest trick
HALF_PI = math.pi / 2.0

NQT = S // 128             # 16 q/k tiles per sequence
# triangular packing offsets for P_T: row kt covers q in [kt*128, S)
OFFS = [0] * NQT
for _kt in range(1, NQT):
    OFFS[_kt] = OFFS[_kt - 1] + (S - (_kt - 1) * 128)
PTRI_W = OFFS[-1] + (S - (NQT - 1) * 128)   # 17408

TC = 512                   # token chunk for projections
NTC = T // TC              # 16
CPB = S // TC              # proj chunks per batch = 4
QC = 512                   # P@V q-chunk width
NPV = S // QC              # P@V chunks per problem = 4
SGW = 1024                 # scores psum group width (2 banks)


def _build_nc():
    nc = bacc.Bacc("TRN2", target_bir_lowering=False, debug=False,
                   num_devices=NCORES)
    dt = mybir.dt

    # ---- I/O ----
    x_in = nc.dram_tensor("x", [NTC, 128, KT * TC], dt.bfloat16,
                          kind="ExternalInput")
    cos_in = nc.dram_tensor("cost", [128, S], dt.bfloat16, kind="ExternalInput")
    sin_in = nc.dram_tensor("sint", [128, S], dt.bfloat16, kind="ExternalInput")
    wq_in = nc.dram_tensor("wq", [D, PC], dt.bfloat16, kind="ExternalInput")
    wk_in = nc.dram_tensor("wk", [D, PC], dt.bfloat16, kind="ExternalInput")
    wv_in = nc.dram_tensor("wv", [D, PC], dt.bfloat16, kind="ExternalInput")
    wo_in = nc.dram_tensor("wo", [PC, D], dt.bfloat16, kind="ExternalInput")
    bq_in = nc.dram_tensor("bq", [PC], dt.float32, kind="ExternalInput")
    bk_in = nc.dram_tensor("bk", [PC], dt.float32, kind="ExternalInput")
    bv_in = nc.dram_tensor("bv", [PC], dt.float32, kind="ExternalInput")
    out_d = nc.dram_tensor("out", [KT, T // 512, 128, 512], dt.bfloat16,
                           kind="ExternalOutput")
    KDEBUG = os.environ.get("KDEBUG") == "1"
    if KDEBUG:
        dbg_q = nc.dram_tensor("dbg_q", [128, T], dt.bfloat16, kind="ExternalOutput")
        dbg_k = nc.dram_tensor("dbg_k", [128, T], dt.bfloat16, kind="ExternalOutput")
        dbg_h = nc.dram_tensor("dbg_h", [128, T], dt.bfloat16, kind="ExternalOutput")
        dbg_pt = nc.dram_tensor("dbg_pt", [128, PTRI_W], dt.bfloat16,
                                kind="ExternalOutput")
        dbg_rr = nc.dram_tensor("dbg_rr", [1, S], dt.float16,
                                kind="ExternalOutput")

    # ---- inline constants ----
    # RT = R.T where (R @ q)[2i] = -q[2i+1], (R @ q)[2i+1] = q[2i],
    # block-diagonal over the 2 stacked heads (structure identical).
    r = np.zeros((PC, PC), dtype=np.float32)
    for h in range(HPC):
        for i in range(HD // 2):
            r[h * HD + 2 * i, h * HD + 2 * i + 1] = -1.0
            r[h * HD + 2 * i + 1, h * HD + 2 * i] = 1.0
    rt_np = np.ascontiguousarray(r.T).astype(BF16)
    # causal mask for diagonal scoresT blocks: keep k_local <= q_local
    mask_np = np.tril(np.ones((128, 128), dtype=np.float32)).T.astype(BF16)
    rt_d = nc.inline_tensor(rt_np, "rt_c")
    ones64_np = np.ones((1, HD), dtype=np.float16)
    ones64_d = nc.inline_tensor(ones64_np, "ones64_c")
    mask_d = nc.inline_tensor(mask_np, "mask_c")

    fp32 = dt.float32
    bf16 = dt.bfloat16
    fp16 = dt.float16

    with tile.TileContext(nc) as tc:
        with (
            tc.tile_pool(name="consts", bufs=1) as consts,
            tc.tile_pool(name="resid", bufs=1) as resid,
            tc.tile_pool(name="xp", bufs=2) as xp,
            tc.tile_pool(name="work", bufs=2) as work,
            tc.tile_pool(name="vst", bufs=2) as vst,
            tc.tile_pool(name="stg", bufs=2) as stg,
            tc.tile_pool(name="ptri", bufs=2) as ptri_pool,
            tc.tile_pool(name="bigps", bufs=2, space="PSUM") as bigps,
            tc.tile_pool(name="pvps", bufs=2, space="PSUM") as pvps,
            tc.tile_pool(name="accps", bufs=2, space="PSUM") as accps,
        ):
            # ---- load constants / weights to SBUF ----
            # x chunk 0 DMA first so the PE can start ASAP; weights on
            # separate queues so wq doesn't queue behind everything.
            xt0 = xp.tile([128, KT, TC], bf16, tag="xt")
            for _kt in range(KT):
                nc.scalar.dma_start(out=xt0[:, _kt, :],
                                    in_=x_in[0, :, ds(_kt * TC, TC)])

            wq_sb = consts.tile([128, KT, PC], bf16, tag="wq")
            wk_sb = consts.tile([128, KT, PC], bf16, tag="wk")
            wv_sb = consts.tile([128, KT, PC], bf16, tag="wv")
            nc.sync.dma_start(
                out=wq_sb, in_=wq_in.ap().rearrange("(kt p) m -> p kt m", p=128))
            nc.gpsimd.dma_start(
                out=wk_sb, in_=wk_in.ap().rearrange("(kt p) m -> p kt m", p=128))
            nc.scalar.dma_start(
                out=wv_sb, in_=wv_in.ap().rearrange("(kt p) m -> p kt m", p=128))
            wo_sb = consts.tile([128, D], bf16, tag="wo")
            nc.gpsimd.dma_start(out=wo_sb, in_=wo_in[:, :])
            rt_sb = consts.tile([128, 128], bf16, tag="rt")
            nc.sync.dma_start(out=rt_sb, in_=rt_d[:, :])
            mask_sb = consts.tile([128, 128], bf16, tag="mask")
            nc.sync.dma_start(out=mask_sb, in_=mask_d[:, :])
            ones64f_sb = consts.tile([1, HD], fp16, tag="ones64")
            nc.sync.dma_start(out=ones64f_sb, in_=ones64_d[:, :])
            bq_sb = consts.tile([128, 1], fp32, tag="bq")
            nc.sync.dma_start(out=bq_sb, in_=bq_in.ap().rearrange("(p o) -> p o", o=1))
            bk_sb = consts.tile([128, 1], fp32, tag="bk")
            nc.sync.dma_start(out=bk_sb, in_=bk_in.ap().rearrange("(p o) -> p o", o=1))
            bv_sb = consts.tile([128, 1], fp32, tag="bv")
            nc.sync.dma_start(out=bv_sb, in_=bv_in.ap().rearrange("(p o) -> p o", o=1))

            # ---- residents ----
            # qT/kT rotate through 2 buffers (proj of batch b+1 overlaps
            # scores of batch b); hT stays per-batch (read by o-proj two
            # problems later).
            qkh = {}

            def get_qk(bb, which):
                if (bb, which) not in qkh:
                    t = resid.tile([128, S], bf16, tag=which, bufs=2,
                                   name=f"{which}{bb}")
                    qkh[(bb, which)] = t
                return qkh[(bb, which)]

            hT = []
            for bb in range(B):
                th = resid.tile([128, S], bf16, tag=f"hT{bb}")
                hT.append(th)
            # v natural as repeating [ones(64) | d_h0(64) | d_h1(64)]
            # 192-col blocks (plus one trailing ones block): head0's P@V
            # lhsT is [ones|d0] (den in psum rows 0-63, h in 64-127) and
            # head1's is [d1|ones-of-next-block] (h in 0-63, den 64-127) --
            # both plain contiguous 128-col slices.
            NTB = NTOK // B            # 16 tok tiles per batch
            VW = NTB * 192 + 64
            vN = []
            for bb in range(B):
                tv = resid.tile([128, VW], bf16, tag=f"vN{bb}")
                vN.append(tv)
                nc.vector.memset(
                    bass.AP(tensor=tv.tensor, offset=tv.offset,
                            ap=[tv.ap[0], [192, NTB + 1], [1, 64]]), 1.0)
            # RoPE cos/sin tables [128, S] bf16, computed host-side
            cos_sb = consts.tile([128, S], bf16, tag="cosT")
            nc.gpsimd.dma_start(out=cos_sb, in_=cos_in[:, :])
            sin_sb = consts.tile([128, S], bf16, tag="sinT")
            nc.gpsimd.dma_start(out=sin_sb, in_=sin_in[:, :])

            # ---- phase 1: QKV projection + RoPE, per token chunk ----
            xt_cache = {0: xt0}

            def get_xt(tci):
                if tci not in xt_cache:
                    xt = xp.tile([128, KT, TC], bf16, tag="xt")
                    nc.scalar.dma_start(out=xt.rearrange("p a b -> p (a b)"),
                                        in_=x_in[tci, :, :])
                    xt_cache[tci] = xt
                return xt_cache[tci]

            def proj_qk_mm(tci, which, st):
                """q/k projection matmuls for one 512-token chunk."""
                xt = get_xt(tci)
                if tci + 1 < NTC:
                    get_xt(tci + 1)
                w_sb, b_sb = ((wq_sb, bq_sb) if which == "q"
                              else (wk_sb, bk_sb))
                pa = accps.tile([128, TC], fp32, tag="acc")
                for kt in range(KT):
                    nc.tensor.matmul(pa, lhsT=w_sb[:, kt, :],
                                     rhs=xt[:, kt, :],
                                     start=(kt == 0), stop=(kt == KT - 1))
                a_sb = work.tile([128, TC], bf16, tag="a_sb", bufs=4)
                nc.vector.tensor_scalar_add(a_sb, pa, b_sb)
                st[which] = a_sb

            def proj_qk_rope(tci, which, st):
                """RoPE for the chunk: rotation matmul + combine."""
                dest = get_qk(tci // CPB, which)
                a_sb = st.pop(which)
                pb = accps.tile([128, TC], fp32, tag="acc")
                nc.tensor.matmul(pb, lhsT=rt_sb, rhs=a_sb,
                                 start=True, stop=True)
                ssl = ds((tci * TC) % S, TC)
                t1 = work.tile([128, TC], bf16, tag="t1", bufs=4)
                nc.vector.tensor_mul(t1, a_sb, cos_sb[:, ssl])
                t2 = work.tile([128, TC], bf16, tag="t2", bufs=4)
                nc.vector.tensor_mul(t2, pb, sin_sb[:, ssl])
                nc.gpsimd.tensor_add(dest[:, ts(tci % CPB, TC)], t1, t2)

            def proj_v(tci):
                """v projection, transposed production (wv stationary),
                then DMA-transposed into natural layout vN on sync."""
                xt = get_xt(tci)
                pv = accps.tile([128, TC], fp32, tag="acc")
                for kt in range(KT):
                    nc.tensor.matmul(pv, lhsT=wv_sb[:, kt, :],
                                     rhs=xt[:, kt, :],
                                     start=(kt == 0), stop=(kt == KT - 1))
                vTst = vst.tile([128, TC], bf16, tag="vTst", bufs=4)
                nc.vector.tensor_scalar_add(vTst, pv, bv_sb)
                for sub in range(TC // 128):
                    tl = (tci % CPB) * (TC // 128) + sub
                    nc.sync.dma_start_transpose(
                        vN[tci // CPB][:, ds(192 * tl + 64, 128)],
                        vTst[:, ts(sub, 128)])

            def proj_pieces(b):
                out = []
                for cc in range(CPB):
                    tci = b * CPB + cc
                    st = {}
                    out.append(lambda t=tci, s=st: proj_qk_mm(t, "q", s))
                    out.append(lambda t=tci, s=st: proj_qk_mm(t, "k", s))
                    out.append(lambda t=tci, s=st: proj_qk_rope(t, "q", s))
                    out.append(lambda t=tci, s=st: proj_qk_rope(t, "k", s))
                    out.append(lambda t=tci: proj_v(t))
                return out

            # ---- phase 2a: scores + exp + mask for one (batch, head) ----
            def score_pieces(i, pt):
                """Closures, each = one <=1024-col psum group: matmul the
                row pieces intersecting the window, exp into pt, mask
                any diagonal blocks fully inside the window."""
                b, h = i // HPC, i % HPC
                hsl = ds(h * HD, HD)
                kTb = get_qk(b, "k")
                qTb = get_qk(b, "q")
                pieces = []
                x0 = 0
                while x0 < PTRI_W:
                    w = min(SGW, PTRI_W - x0)

                    def piece(x0=x0, w=w):
                        sc = bigps.tile([128, SGW], fp32, tag="big")
                        # rows intersecting flat-span window [x0, x0+w)
                        for kt in range(NQT):
                            r0, r1 = OFFS[kt], OFFS[kt] + (S - kt * 128)
                            lo, hi = max(r0, x0), min(r1, x0 + w)
                            if lo >= hi:
                                continue
                            q0 = kt * 128 + (lo - r0)
                            # split at 512-aligned psum columns: a matmul
                            # output must not cross a PSUM bank boundary
                            cuts = [lo]
                            nb = (lo - x0) // 512 * 512 + 512 + x0
                            while nb < hi:
                                cuts.append(nb)
                                nb += 512
                            cuts.append(hi)
                            for aa, bb in zip(cuts, cuts[1:]):
                                nc.tensor.matmul(
                                    sc[:, ds(aa - x0, bb - aa)],
                                    lhsT=kTb[hsl, ds(kt * 128, 128)],
                                    rhs=qTb[hsl, ds(q0 + (aa - lo), bb - aa)],
                                    start=True, stop=True)
                        nc.scalar.activation(
                            pt[:, ds(x0, w)], sc[:, 0:w],
                            mybir.ActivationFunctionType.Exp, scale=SCALE)
                        for kt in range(NQT):
                            if x0 <= OFFS[kt] and OFFS[kt] + 128 <= x0 + w:
                                dsl = ds(OFFS[kt], 128)
                                nc.gpsimd.tensor_mul(
                                    pt[:, dsl], pt[:, dsl], mask_sb)

                    pieces.append(piece)
                    x0 += w
                return pieces

            # ---- phase 2b: P@V (V stationary) + normalize into hT ----
            # Each P@V chunk's psum acc [128,512] (den rows + h rows) is
            # cast once to fp16 staging; den rows gather into [16,128],
            # DMA-transpose to [128,16], DVE reciprocal -> fp16, one DMA
            # scatters to rec_row [1,S] (q-order), and a K=1 ones-matmul
            # broadcasts 1/den to [64,512] psum for the normalize mult.
            def pv_pieces(i, pt):
                b, h = i // HPC, i % HPC
                st = {}
                pieces = []

                def mm_piece(c):
                    q0 = c * QC
                    acc = pvps.tile([128, QC], fp32, tag="pv")
                    kts = [kt for kt in range(NQT) if kt * 128 < q0 + QC]
                    for j, kt in enumerate(kts):
                        lo = max(kt * 128, q0)
                        w = q0 + QC - lo
                        lt = vN[b][:, ds(192 * kt + 128 * h, 128)]
                        nc.tensor.matmul(
                            acc[:, ds(lo - q0, w)], lhsT=lt,
                            rhs=pt[:, ds(OFFS[kt] + lo - kt * 128, w)],
                            start=(j == 0), stop=(j == len(kts) - 1))
                    if c == 0:
                        d16 = stg.tile([16, 128], fp16, tag="den16")
                        st["den16"] = d16
                    sacc = stg.tile([128, QC], fp16, tag="sacc", bufs=8)
                    nc.vector.tensor_copy(sacc, acc)
                    st[("hu", c)] = sacc[ds(64 - 64 * h, HD), :]
                    nc.sync.dma_start(
                        out=st["den16"][ds(4 * c, 4), :],
                        in_=sacc[ds(64 * h, 1), :])

                def rec_piece():
                    # den16 rows are already q-ordered: den16[R, m] =
                    # den(q = R*128 + m). Reciprocal in this layout (16
                    # lanes), then one partition-major DMA linearizes to
                    # rec_row[0, q]. bf16 rec_row: fp32 matmuls run in
                    # LOW_HIGH double-pass mode (~1us for N=512).
                    den32 = stg.tile([16, 128], fp32, tag="den32")
                    nc.vector.tensor_copy(den32, st["den16"])
                    rec32 = stg.tile([16, 128], fp32, tag="rec32")
                    nc.vector.reciprocal_approx_fast(rec32, den32)
                    rec16 = stg.tile([16, 128], fp16, tag="rec16")
                    nc.vector.tensor_copy(rec16, rec32)
                    rr = stg.tile([1, S], fp16, tag="rr", bufs=2)
                    nc.sync.dma_start(
                        out=bass.AP(tensor=rr.tensor, offset=rr.offset,
                                    ap=[[rr.ap[0][0], 1], [1, S]]),
                        in_=rec16[:, :])
                    for c in range(NPV):
                        recb = pvps.tile([HD, QC], fp32, tag="pv")
                        nc.tensor.matmul(
                            recb, lhsT=ones64f_sb[0:1, :],
                            rhs=rr[0:1, ds(c * QC, QC)],
                            start=True, stop=True)
                        nc.vector.tensor_mul(
                            hT[b][ds(h * HD, HD), ds(c * QC, QC)],
                            st[("hu", c)], recb)
                    if KDEBUG and i == 0:
                        nc.sync.dma_start(out=dbg_rr[:, :], in_=rr)

                for c in range(NPV):
                    pieces.append(lambda c=c: mm_piece(c))
                pieces.append(rec_piece)
                return pieces

            # ---- phase 3: output projection (partial, transposed) ----
            def oproj_piece(b, ft, cc, tail=False):
                po = accps.tile([128, 512], fp32, tag="acc")
                nc.tensor.matmul(
                    po, lhsT=wo_sb[:, ts(ft, 128)],
                    rhs=hT[b][:, ds(cc * 512, 512)],
                    start=True, stop=True)
                ostage = work.tile([128, 512], bf16, tag="ostage", bufs=4)
                nc.vector.tensor_copy(ostage, po)
                nc.gpsimd.dma_start(
                    out=out_d[ft, b * (S // 512) + cc, :, :], in_=ostage)

            def oproj_pieces(b, fts, tail=False):
                return [lambda f=ft, c=cc, bb=b: oproj_piece(bb, f, c,
                                                             tail=tail)
                        for ft in fts for cc in range(S // 512)]

            # ---- emission schedule ----
            def interleave(main, others, ratio=None):
                """Emit main pieces with `others` spliced evenly."""
                if ratio is None:
                    ratio = max(1, len(main) // max(1, len(others)))
                oi = 0
                for n, m in enumerate(main):
                    m()
                    if n % ratio == ratio - 1 and oi < len(others):
                        others[oi]()
                        oi += 1
                for o in others[oi:]:
                    o()

            # startup: batch-0 projection
            for p in proj_pieces(0):
                p()

            nprob = B * HPC
            pts = {}
            pvq = []      # pending P@V pieces (from previous problem)
            pending = []  # filler queue, carried across iterations
            for i in range(nprob + 1):
                # projection for batch i//2+1 split over iterations 2b-2,
                # 2b-1; O-proj for batch (i-3)//2 split over 2b+3, 2b+4.
                # Proj pieces go first: their casts feed the PE (rope
                # matmuls) soonest.
                bb = i // 2 + 1
                half = i % 2
                if bb < B:
                    pending += proj_pieces(bb)[10 * half:10 * half + 10]
                pending += pvq
                pvq = []
                if i >= 3:
                    ob = (i - 3) // 2
                    ohalf = (i - 3) % 2
                    pending += oproj_pieces(ob, range(4 * ohalf, 4 * ohalf + 4))
                if i == nprob:   # epilogue: last batch's O-proj
                    pending += oproj_pieces(B - 1, range(8), tail=True)
                if i < nprob:
                    pt = ptri_pool.tile([128, PTRI_W], bf16, tag="pt")
                    pts[i] = pt
                    wins = score_pieces(i, pt)
                    nwin = len(wins)
                    for wi, wpc in enumerate(wins):
                        wpc()
                        # drain fillers proportionally; leftovers carry
                        # into the next iteration instead of dumping here
                        want = (len(pending) + nwin - wi - 1) // (nwin - wi)
                        for _ in range(min(want, 2)):
                            if pending:
                                pending.pop(0)()
                    pvq = pv_pieces(i, pt)
                else:
                    for p in pending:
                        p()
                    pending = []
            # pv of the last problem ran inside the epilogue fillers
            if KDEBUG:
                for bb in range(B):
                    nc.sync.dma_start(out=dbg_q[:, ts(bb, S)],
                                      in_=get_qk(bb, "q"))
                    nc.sync.dma_start(out=dbg_k[:, ts(bb, S)],
                                      in_=get_qk(bb, "k"))
                    nc.sync.dma_start(out=dbg_h[:, ts(bb, S)], in_=hT[bb])

    nc.compile()
    return nc


_NC_CACHE = None


def _get_nc():
    global _NC_CACHE
    if _NC_CACHE is None:
        _NC_CACHE = _build_nc()
    return _NC_CACHE


def build_in_maps(x, positions, Wqkv, bqkv, Wo, bo):
    xT = x.reshape(T, D).T.astype(BF16)            # [D, T]
    # chunk-block layout [tci, p, kt*512]: contiguous 4KB runs per partition
    xblk = np.ascontiguousarray(
        xT.reshape(KT, 128, NTC, TC).transpose(2, 1, 0, 3).reshape(NTC, 128, KT * TC))
    # RoPE tables, layout matched to the stacked-heads partition dim:
    # partition p -> head-local pair i = (p % 64) // 2
    i_of_p = (np.arange(PC) % HD) // 2
    invf = 1.0 / (ROPE_THETA ** (2.0 * i_of_p / HD))          # [128]
    ang = invf[:, None] * np.asarray(positions[0])[None, :]   # [128, S]
    cost = np.ascontiguousarray(np.cos(ang)).astype(BF16)
    sint = np.ascontiguousarray(np.sin(ang)).astype(BF16)
    in_maps = []
    for c in range(NCORES):
        r0 = c * PC
        wq = np.ascontiguousarray(Wqkv[r0:r0 + PC, :].T).astype(BF16)
        wk = np.ascontiguousarray(Wqkv[D + r0:D + r0 + PC, :].T).astype(BF16)
        wv = np.ascontiguousarray(Wqkv[2 * D + r0:2 * D + r0 + PC, :].T).astype(BF16)
        wo = np.ascontiguousarray(Wo[:, r0:r0 + PC].T).astype(BF16)
        in_maps.append({
            "x": xblk, "cost": cost, "sint": sint,
            "wq": wq, "wk": wk, "wv": wv, "wo": wo,
            "bq": bqkv[r0:r0 + PC].astype(np.float32),
            "bk": bqkv[D + r0:D + r0 + PC].astype(np.float32),
            "bv": bqkv[2 * D + r0:2 * D + r0 + PC].astype(np.float32),
        })
    return in_maps


def assemble_out(res, bo):
    acc = res.results[0]["out"].astype(np.float32)
    for c in range(1, NCORES):
        acc += res.results[c]["out"].astype(np.float32)
    # [KT, T//512, 128, 512] -> [D, T]
    full = acc.transpose(0, 2, 1, 3).reshape(D, T)
    out = full + bo[:, None].astype(np.float32)
    return np.ascontiguousarray(out.T).reshape(B, S, D)


def kernel(x, positions, Wqkv, bqkv, Wo, bo):
    x = np.asarray(x)
    positions = np.asarray(positions)
    Wqkv = np.asarray(Wqkv)
    bqkv = np.asarray(bqkv)
    Wo = np.asarray(Wo)
    bo = np.asarray(bo)
    nc = _get_nc()
    in_maps = build_in_maps(x, positions, Wqkv, bqkv, Wo, bo)
    res = run_bass_kernel_spmd(nc, in_maps, core_ids=list(range(NCORES)))
    return assemble_out(res, bo)
